# revision 23
# baseline (speedup 1.0000x reference)
"""Trainium2 Bass kernel for the DGN-critic GNN message-passing module.

Contract: kernel(**inputs) takes the FULL unsharded inputs (as produced by
setup_inputs) and returns (value, a_w) matching reference().

Strategy: pure data parallel over B=1024 across 8 NeuronCores (128 batch
elems per core).  Within a core, batch elements are processed in groups of
4 so dense matmuls stream 512-wide.  Activations are kept feature-major
([feat, token]) so weights act as the stationary matmul operand and bias+
relu fold into the PSUM->SBUF copy.  Encoder/attention run in bf16 (fp32
accumulation); the critic MLP runs in float32r to keep the tiny `value`
output accurate.  Masking/softmax math is exact fp32.
"""

import numpy as np
import ml_dtypes

import concourse.bass as bass
import concourse.mybir as mybir
import concourse.tile as tile
from concourse.bass_utils import run_bass_kernel_spmd

F32 = mybir.dt.float32
F32R = mybir.dt.float32r
BF16 = mybir.dt.bfloat16
NPBF = ml_dtypes.bfloat16

NCORES = 8
B, N, OBS, H = 1024, 128, 64, 256
BE = B // NCORES          # batch elems per core
G = 4                     # elems per group (512-wide free dim)
NG = BE // G
BIG = 9e15
ESHIFT = -12.0            # constant softmax shift (scores observed <= ~10)


def split_multi_waits(nc):
    """Walrus in this container accepts at most ONE sync wait per
    instruction.  Hoist extra waits onto same-engine NOPs placed just
    before the instruction."""
    main_ctx = nc.cur_bb
    main_bb = main_ctx.bb
    for bbname, bbctx in list(nc.bb_map.items()):
        bb = bbctx.bb if hasattr(bbctx, "bb") else bbctx
        insts = list(bb.instructions)
        if not any(
            i.sync_info and i.sync_info.on_wait and len(i.sync_info.on_wait) > 1
            for i in insts
        ):
            continue
        new_list = []
        for inst in insts:
            si = inst.sync_info
            waits = list(si.on_wait) if si and si.on_wait else []
            if len(waits) > 1:
                for w in waits[:-1]:
                    nop = nc.engines[inst.engine].nop(nofuse=True)
                    nop_inst = (
                        nc.inst_map[nop.ins] if isinstance(nop.ins, str) else nop.ins
                    )
                    # nop() appended itself to the current bb; remove it.
                    lst = main_bb.instructions
                    lst = [i for i in lst if i.name != nop_inst.name]
                    main_bb.instructions = lst
                    nop_inst.sync_info = mybir.SyncInfo(on_wait=[w], on_update=[])
                    new_list.append(nop_inst)
                inst.sync_info = mybir.SyncInfo(
                    on_wait=[waits[-1]], on_update=list(si.on_update or [])
                )
            new_list.append(inst)
        bb.instructions = new_list


def build_program():
    nc = bass.Bass()

    # ---- DRAM I/O (per-core shard) ----
    xt_d = nc.dram_tensor("xt", [BE, OBS, N], BF16, kind="ExternalInput")
    mask_d = nc.dram_tensor("mask", [BE, N, N], F32, kind="ExternalInput")
    act_d = nc.dram_tensor("action", [1, BE * N], BF16, kind="ExternalInput")
    # weights, host-prepacked
    wet_d = nc.dram_tensor("wet", [OBS, H], BF16, kind="ExternalInput")
    wqt_d = nc.dram_tensor("wqt", [128, 2, H], BF16, kind="ExternalInput")
    wkt_d = nc.dram_tensor("wkt", [128, 2, H], BF16, kind="ExternalInput")
    wvt_d = nc.dram_tensor("wvt", [128, 2, H], BF16, kind="ExternalInput")
    bvrow_d = nc.dram_tensor("bvrow", [1, H], BF16, kind="ExternalInput")
    w1t_d = nc.dram_tensor("w1t", [128, 2, H], BF16, kind="ExternalInput")
    w1l_d = nc.dram_tensor("w1l", [1, H], BF16, kind="ExternalInput")
    w2t_d = nc.dram_tensor("w2t", [128, 2, H], BF16, kind="ExternalInput")
    w3t_d = nc.dram_tensor("w3t", [128, 2, H], F32R, kind="ExternalInput")
    w4t_d = nc.dram_tensor("w4t", [128, 2], F32R, kind="ExternalInput")
    be_d = nc.dram_tensor("be2", [128, 2], F32, kind="ExternalInput")
    bq_d = nc.dram_tensor("bq2", [128, 2], F32, kind="ExternalInput")
    bk_d = nc.dram_tensor("bk2", [128, 2], F32, kind="ExternalInput")
    b1_d = nc.dram_tensor("b12", [128, 2], F32, kind="ExternalInput")
    b2_d = nc.dram_tensor("b22", [128, 2], F32, kind="ExternalInput")
    b3_d = nc.dram_tensor("b32", [128, 2], F32, kind="ExternalInput")
    ident_d = nc.dram_tensor("ident", [128, 128], BF16, kind="ExternalInput")
    ones_d = nc.dram_tensor("ones", [1, 128], BF16, kind="ExternalInput")

    aw_d = nc.dram_tensor("a_w", [BE, N, N], F32, kind="ExternalOutput")
    val_d = nc.dram_tensor("value", [1, BE * N], F32, kind="ExternalOutput")

    b4_f = None  # bias b4 is passed via host fold (scalar) - set in kernel()

    Relu = mybir.ActivationFunctionType.Relu
    Copy = mybir.ActivationFunctionType.Copy
    Exp = mybir.ActivationFunctionType.Exp
    Alu = mybir.AluOpType

    from contextlib import ExitStack

    with tile.TileContext(nc) as tc, ExitStack() as ctx:
        const = ctx.enter_context(tc.tile_pool(name="const", bufs=1))
        io = ctx.enter_context(tc.tile_pool(name="io", bufs=4))
        wk = ctx.enter_context(tc.tile_pool(name="wk", bufs=3))
        awp = ctx.enter_context(tc.tile_pool(name="awp", bufs=3))
        pmm = ctx.enter_context(tc.tile_pool(name="pmm", bufs=5, space="PSUM"))
        psm = ctx.enter_context(tc.tile_pool(name="psm", bufs=3, space="PSUM"))

        # ---- preload constants ----
        def load_const(d, shape, dtype, name):
            t = const.tile(shape, dtype, tag=name)
            nc.sync.dma_start(t[:], d[:])
            return t

        wet = load_const(wet_d, [OBS, H], BF16, "wet")
        wqt = load_const(wqt_d, [128, 2, H], BF16, "wqt")
        wkt = load_const(wkt_d, [128, 2, H], BF16, "wkt")
        wvt = load_const(wvt_d, [128, 2, H], BF16, "wvt")
        bvrow = load_const(bvrow_d, [1, H], BF16, "bvrow")
        w1t = load_const(w1t_d, [128, 2, H], BF16, "w1t")
        w1l = load_const(w1l_d, [1, H], BF16, "w1l")
        w2t = load_const(w2t_d, [128, 2, H], BF16, "w2t")
        w3t = load_const(w3t_d, [128, 2, H], F32R, "w3t")
        w4t = load_const(w4t_d, [128, 2], F32R, "w4t")
        be2 = load_const(be_d, [128, 2], F32, "be2")
        bq2 = load_const(bq_d, [128, 2], F32, "bq2")
        bk2 = load_const(bk_d, [128, 2], F32, "bk2")
        b12 = load_const(b1_d, [128, 2], F32, "b12")
        b22 = load_const(b2_d, [128, 2], F32, "b22")
        b32 = load_const(b3_d, [128, 2], F32, "b32")
        ident = load_const(ident_d, [128, 128], BF16, "ident")
        ones1 = load_const(ones_d, [1, 128], BF16, "ones1")
        actt = load_const(act_d, [1, BE * N], BF16, "actt")
        eshift = const.tile([128, 1], F32, tag="eshift")
        nc.vector.memset(eshift[:], ESHIFT)

        def seg_load(g):
            g4 = g * G
            st = {}
            xT = io.tile([OBS, G, 128], BF16, tag="xT")
            nc.sync.dma_start(
                out=xT[:], in_=xt_d[g4 : g4 + G].rearrange("e d n -> d e n")
            )
            mask_g = io.tile([128, G, N], F32, tag="mask_g")
            nc.sync.dma_start(
                out=mask_g[:], in_=mask_d[g4 : g4 + G].rearrange("e n m -> n e m")
            )
            st["mask_g"] = mask_g
            st["xT"] = xT
            return st

        def seg_h1(g, st):
            xT = st["xT"]
            h1 = wk.tile([128, 2, 512], BF16, tag="h1")
            for m in range(2):
                ps = pmm.tile([128, 512], F32, tag="mm")
                nc.tensor.matmul(
                    ps[:],
                    wet[:, m * 128 : (m + 1) * 128],
                    xT.rearrange("d e n -> d (e n)"),
                    start=True,
                    stop=True,
                )
                nc.scalar.activation(
                    h1[:, m, :], ps[:], Relu, bias=be2[:, m : m + 1], scale=1.0
                )
            st["h1"] = h1

        def _qk_layer(h1, wt, bias2, tag):
            o = wk.tile([128, 2, 512], BF16, tag=tag)
            for m in range(2):
                ps = pmm.tile([128, 512], F32, tag="mm")
                for kt in range(2):
                    nc.tensor.matmul(
                        ps[:],
                        wt[:, kt, m * 128 : (m + 1) * 128],
                        h1[:, kt, :],
                        start=(kt == 0),
                        stop=(kt == 1),
                    )
                nc.scalar.activation(
                    o[:, m, :], ps[:], Relu, bias=bias2[:, m : m + 1], scale=1.0
                )
            return o

        def seg_q(g, st):
            st["qT"] = _qk_layer(st["h1"], wqt, bq2, "qT")

        def seg_k(g, st):
            st["kT"] = _qk_layer(st["h1"], wkt, bk2, "kT")

        def seg_v(g, st):
            h1 = st["h1"]
            v_bf = wk.tile([128, G, H], BF16, tag="v_bf")
            for e in range(G):
                ps = psm.tile([128, H], F32, tag="sm")
                for kt in range(2):
                    nc.tensor.matmul(
                        ps[:],
                        h1[:, kt, e * 128 : (e + 1) * 128],
                        wvt[:, kt, :],
                        start=(kt == 0),
                        stop=False,
                    )
                nc.tensor.matmul(ps[:], ones1[:], bvrow[:], start=False, stop=True)
                nc.vector.tensor_scalar(
                    out=v_bf[:, e, :], in0=ps[:], scalar1=0.0, scalar2=None,
                    op0=Alu.max,
                )
            st["v_bf"] = v_bf

        def seg_scores(g, st):
            g4 = g * G
            qT, kT, mask_g = st["qT"], st["kT"], st["mask_g"]
            nb = wk.tile([128, G * N], F32, tag="nb")
            nc.gpsimd.tensor_scalar(
                out=nb[:], in0=mask_g.rearrange("n e m -> n (e m)"),
                scalar1=BIG, scalar2=-BIG, op0=Alu.mult, op1=Alu.add,
            )
            t_g = wk.tile([128, G, N], F32, tag="t_g")
            for e in range(G):
                sps = psm.tile([128, N], F32, tag="sm")
                for kt in range(2):
                    nc.tensor.matmul(
                        sps[:],
                        qT[:, kt, e * 128 : (e + 1) * 128],
                        kT[:, kt, e * 128 : (e + 1) * 128],
                        start=(kt == 0),
                        stop=(kt == 1),
                    )
                nc.vector.scalar_tensor_tensor(
                    out=t_g[:, e, :], in0=sps[:], scalar=0.0, in1=mask_g[:, e, :],
                    op0=Alu.max, op1=Alu.mult,
                )
            a_w = awp.tile([128, G, N], F32, tag="a_w")
            nc.gpsimd.tensor_add(
                a_w.rearrange("n e m -> n (e m)"),
                t_g.rearrange("n e m -> n (e m)"),
                nb[:],
            )
            nc.sync.dma_start(
                out=aw_d[g4 : g4 + G].rearrange("e n m -> n e m"), in_=a_w[:]
            )
            E_g = wk.tile([128, G, N], BF16, tag="E_g")
            nc.scalar.activation(
                E_g.rearrange("n e m -> n (e m)"),
                a_w.rearrange("n e m -> n (e m)"),
                Exp, bias=eshift[:], scale=1.0,
            )
            ssum = wk.tile([128, G], F32, tag="ssum")
            nc.vector.tensor_reduce(
                ssum[:], E_g[:], axis=mybir.AxisListType.X, op=Alu.add
            )
            rinv = wk.tile([128, G], F32, tag="rinv")
            nc.vector.reciprocal(rinv[:], ssum[:])
            att = wk.tile([128, G, N], BF16, tag="att")
            for e in range(G):
                nc.vector.tensor_scalar(
                    out=att[:, e, :], in0=E_g[:, e, :],
                    scalar1=rinv[:, e : e + 1], scalar2=None, op0=Alu.mult,
                )
            st["att"] = att

        def seg_attT(g, st):
            att = st["att"]
            aT = wk.tile([128, G * 128], BF16, tag="aT")
            for e in range(G):
                nc.sync.dma_start(
                    out=aT[:, e * 128 : (e + 1) * 128], in_=att[:, e, :],
                    transpose=True,
                )
            st["aT"] = aT

        def seg_h2(g, st):
            v_bf, aT = st["v_bf"], st["aT"]
            h2 = wk.tile([128, 2, 512], BF16, tag="h2")
            for m in range(2):
                ps = pmm.tile([128, 512], F32, tag="mm")
                for e in range(G):
                    nc.tensor.matmul(
                        ps[:, e * 128 : (e + 1) * 128],
                        v_bf[:, e, m * 128 : (m + 1) * 128],
                        aT[:, e * 128 : (e + 1) * 128],
                        start=True,
                        stop=True,
                    )
                nc.scalar.activation(h2[:, m, :], ps[:], Copy)
            st["h2"] = h2

        def _critic_layer(g, inp, wt, bias2, tag, extra=None, act_copies=(),
                          odt=F32R):
            o = wk.tile([128, 2, 512], odt, tag=tag)
            for m in range(2):
                ps = pmm.tile([128, 512], F32, tag="mm")
                nfin = 2 if extra is None else 3
                idx = 0
                for kt in range(2):
                    idx += 1
                    nc.tensor.matmul(
                        ps[:],
                        wt[:, kt, m * 128 : (m + 1) * 128],
                        inp[:, kt, :],
                        start=(kt == 0),
                        stop=(idx == nfin),
                    )
                if extra is not None:
                    w1l_t, act_slab = extra
                    idx += 1
                    nc.tensor.matmul(
                        ps[:],
                        w1l_t[:, m * 128 : (m + 1) * 128],
                        act_slab,
                        start=False,
                        stop=(idx == nfin),
                    )
                if m in act_copies:
                    nc.scalar.activation(
                        o[:, m, :], ps[:], Relu,
                        bias=bias2[:, m : m + 1], scale=1.0,
                    )
                else:
                    nc.vector.tensor_scalar(
                        out=o[:, m, :], in0=ps[:],
                        scalar1=bias2[:, m : m + 1], scalar2=0.0,
                        op0=Alu.add, op1=Alu.max,
                    )
            return o

        def seg_c2(g, st):
            g4 = g * G
            act_slab = actt[:, g4 * N : (g4 + G) * N]
            st["c2"] = _critic_layer(
                g, st["h2"], w1t, b12, "c2", extra=(w1l, act_slab), odt=BF16
            )

        def seg_c3(g, st):
            st["c3"] = _critic_layer(g, st["c2"], w2t, b22, "c3")

        def seg_c4(g, st):
            st["c4"] = _critic_layer(g, st["c3"], w3t, b32, "c4", act_copies=(0,))

        def seg_value(g, st):
            c4 = st["c4"]
            vps = psm.tile([1, 512], F32, tag="sm")
            for kt in range(2):
                nc.tensor.matmul(
                    vps[:],
                    w4t[:, kt : kt + 1],
                    c4[:, kt, :],
                    start=(kt == 0),
                    stop=(kt == 1),
                )
            val_sb = wk.tile([1, 512], F32, tag="val_sb")
            nc.scalar.activation(val_sb[:], vps[:], Copy)
            nc.sync.dma_start(
                out=val_d[:, g * 512 : (g + 1) * 512], in_=val_sb[:]
            )

        # 3-deep software pipeline: per outer step, interleave segments of
        # groups a=t (fresh), b=t-1 (attention tail), c=t-2 (critic tail)
        # so every cross-engine handoff is covered by PE work from another
        # group.
        state = {0: seg_load(0)}
        for t in range(NG + 2):
            a, bq, cq = t, t - 1, t - 2
            ina = a < NG
            inb = 0 <= bq < NG
            inc_ = 0 <= cq
            if a + 1 < NG:
                state[a + 1] = seg_load(a + 1)
            if ina:
                seg_h1(a, state[a])
            if inc_:
                seg_c3(cq, state[cq])
            if inb:
                seg_attT(bq, state[bq])
            if ina:
                seg_q(a, state[a])
            if inc_:
                seg_c4(cq, state[cq])
            if ina:
                seg_k(a, state[a])
            if inb:
                seg_h2(bq, state[bq])
            if inc_:
                seg_value(cq, state.pop(cq))
            if ina:
                seg_v(a, state[a])
            if inb:
                seg_c2(bq, state[bq])
            if ina:
                seg_scores(a, state[a])

    split_multi_waits(nc)
    return nc


_CACHE = {}


def _get_program():
    if "nc" not in _CACHE:
        _CACHE["nc"] = build_program()
    return _CACHE["nc"]


def _pack_kt(wT):
    """[256, out] -> [128, 2, out] with kt-major partition packing."""
    out = wT.shape[1]
    return np.ascontiguousarray(
        wT.reshape(2, 128, out).transpose(1, 0, 2)
    )


def _pack_bias(b):
    return np.ascontiguousarray(b.reshape(2, 128).T)


def kernel(**inputs):
    x = np.asarray(inputs["x"], np.float32)
    mask = np.asarray(inputs["mask"], np.float32)
    action = np.asarray(inputs["action"], np.float32)
    We = np.asarray(inputs["We"], np.float32)
    be_ = np.asarray(inputs["be"], np.float32)
    Wv = np.asarray(inputs["Wv"], np.float32)
    bv = np.asarray(inputs["bv"], np.float32)
    Wk = np.asarray(inputs["Wk"], np.float32)
    bk = np.asarray(inputs["bk"], np.float32)
    Wq = np.asarray(inputs["Wq"], np.float32)
    bq = np.asarray(inputs["bq"], np.float32)
    W1 = np.asarray(inputs["W1"], np.float32)
    b1 = np.asarray(inputs["b1"], np.float32)
    W2 = np.asarray(inputs["W2"], np.float32)
    b2 = np.asarray(inputs["b2"], np.float32)
    W3 = np.asarray(inputs["W3"], np.float32)
    b3 = np.asarray(inputs["b3"], np.float32)
    W4 = np.asarray(inputs["W4"], np.float32)
    b4 = np.asarray(inputs["b4"], np.float32)

    consts = dict(
        wet=np.ascontiguousarray(We.T).astype(NPBF),
        wqt=_pack_kt(Wq.T).astype(NPBF),
        wkt=_pack_kt(Wk.T).astype(NPBF),
        wvt=_pack_kt(Wv.T).astype(NPBF),
        bvrow=bv.reshape(1, H).astype(NPBF),
        w1t=_pack_kt(np.ascontiguousarray(W1[:, :H].T)),
        w1l=np.ascontiguousarray(W1[:, H]).reshape(1, H),
        w2t=_pack_kt(np.ascontiguousarray(W2.T)),
        w3t=_pack_kt(np.ascontiguousarray(W3.T)),
        w4t=np.ascontiguousarray(W4.reshape(256).reshape(2, 128).T),
        be2=_pack_bias(be_),
        bq2=_pack_bias(bq),
        bk2=_pack_bias(bk),
        b12=_pack_bias(b1),
        b22=_pack_bias(b2),
        b32=_pack_bias(b3),
        ident=np.eye(128, dtype=np.float32).astype(NPBF),
        ones=np.ones((1, 128), dtype=np.float32).astype(NPBF),
    )
    for k in ("w1t", "w1l", "w2t"):
        consts[k] = consts[k].astype(NPBF)
    for k in ("w3t", "w4t"):
        consts[k] = consts[k].astype(np.float32)

    xt = np.ascontiguousarray(x.transpose(0, 2, 1)).astype(NPBF)
    in_maps = []
    for c in range(NCORES):
        sl = slice(c * BE, (c + 1) * BE)
        m = dict(consts)
        m["xt"] = xt[sl]
        m["mask"] = np.ascontiguousarray(mask[sl])
        m["action"] = np.ascontiguousarray(
            action[sl].reshape(1, BE * N)
        ).astype(NPBF)
        in_maps.append(m)

    nc = _get_program()
    res = run_bass_kernel_spmd(nc, in_maps, list(range(NCORES)))
    _CACHE["last_res"] = res

    aw_parts = []
    val_parts = []
    for c in range(NCORES):
        aw_parts.append(res.results[c]["a_w"])
        val_parts.append(res.results[c]["value"].reshape(BE, N, 1))
    a_w = np.concatenate(aw_parts, axis=0)
    value = np.concatenate(val_parts, axis=0) + b4.reshape(1, 1, 1)
    return value.astype(np.float32), a_w.astype(np.float32)


# revision 24
# speedup vs baseline: 1.1671x; 1.1671x over previous
"""Trainium2 Bass kernel for the DGN-critic GNN message-passing module.

Contract: kernel(**inputs) takes the FULL unsharded inputs (as produced by
setup_inputs) and returns (value, a_w) matching reference().

Strategy: pure data parallel over B=1024 across 8 NeuronCores (128 batch
elems per core).  Within a core, batch elements are processed in groups of
4 so dense matmuls stream 512-wide.  Activations are kept feature-major
([feat, token]) so weights act as the stationary matmul operand and bias+
relu fold into the PSUM->SBUF copy.  Encoder/attention run in bf16 (fp32
accumulation); the critic MLP runs in float32r to keep the tiny `value`
output accurate.  Masking/softmax math is exact fp32.
"""

import numpy as np
import ml_dtypes

import concourse.bass as bass
import concourse.mybir as mybir
import concourse.tile as tile
from concourse.bass_utils import run_bass_kernel_spmd

F32 = mybir.dt.float32
F32R = mybir.dt.float32r
BF16 = mybir.dt.bfloat16
NPBF = ml_dtypes.bfloat16

NCORES = 8
B, N, OBS, H = 1024, 128, 64, 256
BE = B // NCORES          # batch elems per core
G = 4                     # elems per group (512-wide free dim)
NG = BE // G
BIG = 9e15
ESHIFT = -12.0            # constant softmax shift (scores observed <= ~10)


def split_multi_waits(nc):
    """Walrus in this container accepts at most ONE sync wait per
    instruction.  Hoist extra waits onto same-engine NOPs placed just
    before the instruction."""
    main_ctx = nc.cur_bb
    main_bb = main_ctx.bb
    for bbname, bbctx in list(nc.bb_map.items()):
        bb = bbctx.bb if hasattr(bbctx, "bb") else bbctx
        insts = list(bb.instructions)
        if not any(
            i.sync_info and i.sync_info.on_wait and len(i.sync_info.on_wait) > 1
            for i in insts
        ):
            continue
        new_list = []
        for inst in insts:
            si = inst.sync_info
            waits = list(si.on_wait) if si and si.on_wait else []
            if len(waits) > 1:
                for w in waits[:-1]:
                    nop = nc.engines[inst.engine].nop(nofuse=True)
                    nop_inst = (
                        nc.inst_map[nop.ins] if isinstance(nop.ins, str) else nop.ins
                    )
                    # nop() appended itself to the current bb; remove it.
                    lst = main_bb.instructions
                    lst = [i for i in lst if i.name != nop_inst.name]
                    main_bb.instructions = lst
                    nop_inst.sync_info = mybir.SyncInfo(on_wait=[w], on_update=[])
                    new_list.append(nop_inst)
                inst.sync_info = mybir.SyncInfo(
                    on_wait=[waits[-1]], on_update=list(si.on_update or [])
                )
            new_list.append(inst)
        bb.instructions = new_list


def build_program():
    nc = bass.Bass()

    # ---- DRAM I/O (per-core shard) ----
    xt_d = nc.dram_tensor("xt", [BE, OBS, N], BF16, kind="ExternalInput")
    mask_d = nc.dram_tensor("mask", [BE, N, N], F32, kind="ExternalInput")
    act_d = nc.dram_tensor("action", [1, BE * N], BF16, kind="ExternalInput")
    # weights, host-prepacked
    wet_d = nc.dram_tensor("wet", [OBS, H], BF16, kind="ExternalInput")
    wqt_d = nc.dram_tensor("wqt", [128, 2, H], BF16, kind="ExternalInput")
    wkt_d = nc.dram_tensor("wkt", [128, 2, H], BF16, kind="ExternalInput")
    wvt_d = nc.dram_tensor("wvt", [128, 2, H], BF16, kind="ExternalInput")
    bvrow_d = nc.dram_tensor("bvrow", [1, H], BF16, kind="ExternalInput")
    w1t_d = nc.dram_tensor("w1t", [128, 2, H], BF16, kind="ExternalInput")
    w1l_d = nc.dram_tensor("w1l", [1, H], BF16, kind="ExternalInput")
    w2t_d = nc.dram_tensor("w2t", [128, 2, H], BF16, kind="ExternalInput")
    w3t_d = nc.dram_tensor("w3t", [128, 2, H], F32R, kind="ExternalInput")
    w4t_d = nc.dram_tensor("w4t", [128, 2], F32R, kind="ExternalInput")
    be_d = nc.dram_tensor("be2", [128, 2], F32, kind="ExternalInput")
    bq_d = nc.dram_tensor("bq2", [128, 2], F32, kind="ExternalInput")
    bk_d = nc.dram_tensor("bk2", [128, 2], F32, kind="ExternalInput")
    b1_d = nc.dram_tensor("b12", [128, 2], F32, kind="ExternalInput")
    b2_d = nc.dram_tensor("b22", [128, 2], F32, kind="ExternalInput")
    b3_d = nc.dram_tensor("b32", [128, 2], F32, kind="ExternalInput")
    ident_d = nc.dram_tensor("ident", [128, 128], BF16, kind="ExternalInput")
    ones_d = nc.dram_tensor("ones", [1, 128], BF16, kind="ExternalInput")

    aw_d = nc.dram_tensor("a_w", [BE, N, N], F32, kind="ExternalOutput")
    val_d = nc.dram_tensor("value", [1, BE * N], F32, kind="ExternalOutput")

    b4_f = None  # bias b4 is passed via host fold (scalar) - set in kernel()

    Relu = mybir.ActivationFunctionType.Relu
    Copy = mybir.ActivationFunctionType.Copy
    Exp = mybir.ActivationFunctionType.Exp
    Alu = mybir.AluOpType

    from contextlib import ExitStack

    with tile.TileContext(nc) as tc, ExitStack() as ctx:
        const = ctx.enter_context(tc.tile_pool(name="const", bufs=1))
        io = ctx.enter_context(tc.tile_pool(name="io", bufs=4))
        wk = ctx.enter_context(tc.tile_pool(name="wk", bufs=3))
        awp = ctx.enter_context(tc.tile_pool(name="awp", bufs=3))
        pmm = ctx.enter_context(tc.tile_pool(name="pmm", bufs=5, space="PSUM"))
        psm = ctx.enter_context(tc.tile_pool(name="psm", bufs=3, space="PSUM"))

        # ---- preload constants ----
        def load_const(d, shape, dtype, name):
            t = const.tile(shape, dtype, tag=name)
            nc.sync.dma_start(t[:], d[:])
            return t

        wet = load_const(wet_d, [OBS, H], BF16, "wet")
        wqt = load_const(wqt_d, [128, 2, H], BF16, "wqt")
        wkt = load_const(wkt_d, [128, 2, H], BF16, "wkt")
        wvt = load_const(wvt_d, [128, 2, H], BF16, "wvt")
        bvrow = load_const(bvrow_d, [1, H], BF16, "bvrow")
        w1t = load_const(w1t_d, [128, 2, H], BF16, "w1t")
        w1l = load_const(w1l_d, [1, H], BF16, "w1l")
        w2t = load_const(w2t_d, [128, 2, H], BF16, "w2t")
        w3t = load_const(w3t_d, [128, 2, H], F32R, "w3t")
        w4t = load_const(w4t_d, [128, 2], F32R, "w4t")
        be2 = load_const(be_d, [128, 2], F32, "be2")
        bq2 = load_const(bq_d, [128, 2], F32, "bq2")
        bk2 = load_const(bk_d, [128, 2], F32, "bk2")
        b12 = load_const(b1_d, [128, 2], F32, "b12")
        b22 = load_const(b2_d, [128, 2], F32, "b22")
        b32 = load_const(b3_d, [128, 2], F32, "b32")
        ident = load_const(ident_d, [128, 128], BF16, "ident")
        ones1 = load_const(ones_d, [1, 128], BF16, "ones1")
        actt = load_const(act_d, [1, BE * N], BF16, "actt")
        eshift = const.tile([128, 1], F32, tag="eshift")
        nc.vector.memset(eshift[:], ESHIFT)

        def seg_load(g):
            g4 = g * G
            st = {}
            xT = io.tile([OBS, G, 128], BF16, tag="xT")
            nc.sync.dma_start(
                out=xT[:], in_=xt_d[g4 : g4 + G].rearrange("e d n -> d e n")
            )
            mask_g = io.tile([128, G, N], F32, tag="mask_g")
            nc.sync.dma_start(
                out=mask_g[:], in_=mask_d[g4 : g4 + G].rearrange("e n m -> n e m")
            )
            st["mask_g"] = mask_g
            st["xT"] = xT
            return st

        def seg_h1(g, st):
            xT = st["xT"]
            h1 = wk.tile([128, 2, 512], BF16, tag="h1")
            for m in range(2):
                ps = pmm.tile([128, 512], F32, tag="mm")
                nc.tensor.matmul(
                    ps[:],
                    wet[:, m * 128 : (m + 1) * 128],
                    xT.rearrange("d e n -> d (e n)"),
                    start=True,
                    stop=True,
                )
                nc.scalar.activation(
                    h1[:, m, :], ps[:], Relu, bias=be2[:, m : m + 1], scale=1.0
                )
            st["h1"] = h1

        def _qk_layer(h1, wt, bias2, tag):
            o = wk.tile([128, 2, 512], BF16, tag=tag)
            for m in range(2):
                ps = pmm.tile([128, 512], F32, tag="mm")
                for kt in range(2):
                    nc.tensor.matmul(
                        ps[:],
                        wt[:, kt, m * 128 : (m + 1) * 128],
                        h1[:, kt, :],
                        start=(kt == 0),
                        stop=(kt == 1),
                    )
                nc.scalar.activation(
                    o[:, m, :], ps[:], Relu, bias=bias2[:, m : m + 1], scale=1.0
                )
            return o

        def seg_q(g, st):
            st["qT"] = _qk_layer(st["h1"], wqt, bq2, "qT")

        def seg_k(g, st):
            st["kT"] = _qk_layer(st["h1"], wkt, bk2, "kT")

        def seg_v(g, st):
            h1 = st["h1"]
            v_bf = wk.tile([128, G, H], BF16, tag="v_bf")
            for e in range(G):
                ps = psm.tile([128, H], F32, tag="sm")
                for kt in range(2):
                    nc.tensor.matmul(
                        ps[:],
                        h1[:, kt, e * 128 : (e + 1) * 128],
                        wvt[:, kt, :],
                        start=(kt == 0),
                        stop=False,
                    )
                nc.tensor.matmul(ps[:], ones1[:], bvrow[:], start=False, stop=True)
                nc.vector.tensor_scalar(
                    out=v_bf[:, e, :], in0=ps[:], scalar1=0.0, scalar2=None,
                    op0=Alu.max,
                )
            st["v_bf"] = v_bf

        def seg_scores(g, st):
            g4 = g * G
            qT, kT, mask_g = st["qT"], st["kT"], st["mask_g"]
            nb = wk.tile([128, G * N], F32, tag="nb")
            nc.gpsimd.tensor_scalar(
                out=nb[:], in0=mask_g.rearrange("n e m -> n (e m)"),
                scalar1=BIG, scalar2=-BIG, op0=Alu.mult, op1=Alu.add,
            )
            t_g = wk.tile([128, G, N], F32, tag="t_g")
            for e in range(G):
                sps = psm.tile([128, N], F32, tag="sm")
                for kt in range(2):
                    nc.tensor.matmul(
                        sps[:],
                        qT[:, kt, e * 128 : (e + 1) * 128],
                        kT[:, kt, e * 128 : (e + 1) * 128],
                        start=(kt == 0),
                        stop=(kt == 1),
                    )
                nc.vector.scalar_tensor_tensor(
                    out=t_g[:, e, :], in0=sps[:], scalar=0.0, in1=mask_g[:, e, :],
                    op0=Alu.max, op1=Alu.mult,
                )
            a_w = awp.tile([128, G, N], F32, tag="a_w")
            nc.gpsimd.tensor_add(
                a_w.rearrange("n e m -> n (e m)"),
                t_g.rearrange("n e m -> n (e m)"),
                nb[:],
            )
            nc.sync.dma_start(
                out=aw_d[g4 : g4 + G].rearrange("e n m -> n e m"), in_=a_w[:]
            )
            E_g = wk.tile([128, G, N], BF16, tag="E_g")
            nc.scalar.activation(
                E_g.rearrange("n e m -> n (e m)"),
                a_w.rearrange("n e m -> n (e m)"),
                Exp, bias=eshift[:], scale=1.0,
            )
            ssum = wk.tile([128, G], F32, tag="ssum")
            nc.vector.tensor_reduce(
                ssum[:], E_g[:], axis=mybir.AxisListType.X, op=Alu.add
            )
            rinv = wk.tile([128, G], F32, tag="rinv")
            nc.vector.reciprocal(rinv[:], ssum[:])
            att = wk.tile([128, G, N], BF16, tag="att")
            for e in range(G):
                nc.vector.tensor_scalar(
                    out=att[:, e, :], in0=E_g[:, e, :],
                    scalar1=rinv[:, e : e + 1], scalar2=None, op0=Alu.mult,
                )
            st["att"] = att

        def seg_attT(g, st):
            aT_ps = pmm.tile([128, G * 128], BF16, tag="mm")
            att = st["att"]
            for e in range(G):
                nc.tensor.transpose(
                    aT_ps[:, e * 128 : (e + 1) * 128], att[:, e, :], ident[:]
                )
            aT = wk.tile([128, G * 128], BF16, tag="aT")
            nc.scalar.activation(aT[:], aT_ps[:], Copy)
            st["aT"] = aT

        def seg_h2(g, st):
            v_bf, aT = st["v_bf"], st["aT"]
            h2 = wk.tile([128, 2, 512], BF16, tag="h2")
            for m in range(2):
                ps = pmm.tile([128, 512], F32, tag="mm")
                for e in range(G):
                    nc.tensor.matmul(
                        ps[:, e * 128 : (e + 1) * 128],
                        v_bf[:, e, m * 128 : (m + 1) * 128],
                        aT[:, e * 128 : (e + 1) * 128],
                        start=True,
                        stop=True,
                    )
                nc.scalar.activation(h2[:, m, :], ps[:], Copy)
            st["h2"] = h2

        def _critic_layer(g, inp, wt, bias2, tag, extra=None, act_copies=(),
                          odt=F32R):
            o = wk.tile([128, 2, 512], odt, tag=tag)
            for m in range(2):
                ps = pmm.tile([128, 512], F32, tag="mm")
                nfin = 2 if extra is None else 3
                idx = 0
                for kt in range(2):
                    idx += 1
                    nc.tensor.matmul(
                        ps[:],
                        wt[:, kt, m * 128 : (m + 1) * 128],
                        inp[:, kt, :],
                        start=(kt == 0),
                        stop=(idx == nfin),
                    )
                if extra is not None:
                    w1l_t, act_slab = extra
                    idx += 1
                    nc.tensor.matmul(
                        ps[:],
                        w1l_t[:, m * 128 : (m + 1) * 128],
                        act_slab,
                        start=False,
                        stop=(idx == nfin),
                    )
                if m in act_copies:
                    nc.scalar.activation(
                        o[:, m, :], ps[:], Relu,
                        bias=bias2[:, m : m + 1], scale=1.0,
                    )
                else:
                    nc.vector.tensor_scalar(
                        out=o[:, m, :], in0=ps[:],
                        scalar1=bias2[:, m : m + 1], scalar2=0.0,
                        op0=Alu.add, op1=Alu.max,
                    )
            return o

        def seg_c2(g, st):
            g4 = g * G
            act_slab = actt[:, g4 * N : (g4 + G) * N]
            st["c2"] = _critic_layer(
                g, st["h2"], w1t, b12, "c2", extra=(w1l, act_slab), odt=BF16
            )

        def seg_c3(g, st):
            st["c3"] = _critic_layer(g, st["c2"], w2t, b22, "c3")

        def seg_c4(g, st):
            st["c4"] = _critic_layer(g, st["c3"], w3t, b32, "c4", act_copies=(0,))

        def seg_value(g, st):
            c4 = st["c4"]
            vps = psm.tile([1, 512], F32, tag="sm")
            for kt in range(2):
                nc.tensor.matmul(
                    vps[:],
                    w4t[:, kt : kt + 1],
                    c4[:, kt, :],
                    start=(kt == 0),
                    stop=(kt == 1),
                )
            val_sb = wk.tile([1, 512], F32, tag="val_sb")
            nc.scalar.activation(val_sb[:], vps[:], Copy)
            nc.sync.dma_start(
                out=val_d[:, g * 512 : (g + 1) * 512], in_=val_sb[:]
            )

        # 3-deep software pipeline: per outer step, interleave segments of
        # groups a=t (fresh), b=t-1 (attention tail), c=t-2 (critic tail)
        # so every cross-engine handoff is covered by PE work from another
        # group.
        state = {0: seg_load(0)}
        for t in range(NG + 2):
            a, bq, cq = t, t - 1, t - 2
            ina = a < NG
            inb = 0 <= bq < NG
            inc_ = 0 <= cq
            if a + 1 < NG:
                state[a + 1] = seg_load(a + 1)
            if ina:
                seg_h1(a, state[a])
            if inc_:
                seg_c3(cq, state[cq])
            if inb:
                seg_attT(bq, state[bq])
            if ina:
                seg_q(a, state[a])
            if inc_:
                seg_c4(cq, state[cq])
            if ina:
                seg_k(a, state[a])
            if inb:
                seg_h2(bq, state[bq])
            if inc_:
                seg_value(cq, state.pop(cq))
            if ina:
                seg_v(a, state[a])
            if inb:
                seg_c2(bq, state[bq])
            if ina:
                seg_scores(a, state[a])

    split_multi_waits(nc)
    return nc


_CACHE = {}


def _get_program():
    if "nc" not in _CACHE:
        _CACHE["nc"] = build_program()
    return _CACHE["nc"]


def _pack_kt(wT):
    """[256, out] -> [128, 2, out] with kt-major partition packing."""
    out = wT.shape[1]
    return np.ascontiguousarray(
        wT.reshape(2, 128, out).transpose(1, 0, 2)
    )


def _pack_bias(b):
    return np.ascontiguousarray(b.reshape(2, 128).T)


def kernel(**inputs):
    x = np.asarray(inputs["x"], np.float32)
    mask = np.asarray(inputs["mask"], np.float32)
    action = np.asarray(inputs["action"], np.float32)
    We = np.asarray(inputs["We"], np.float32)
    be_ = np.asarray(inputs["be"], np.float32)
    Wv = np.asarray(inputs["Wv"], np.float32)
    bv = np.asarray(inputs["bv"], np.float32)
    Wk = np.asarray(inputs["Wk"], np.float32)
    bk = np.asarray(inputs["bk"], np.float32)
    Wq = np.asarray(inputs["Wq"], np.float32)
    bq = np.asarray(inputs["bq"], np.float32)
    W1 = np.asarray(inputs["W1"], np.float32)
    b1 = np.asarray(inputs["b1"], np.float32)
    W2 = np.asarray(inputs["W2"], np.float32)
    b2 = np.asarray(inputs["b2"], np.float32)
    W3 = np.asarray(inputs["W3"], np.float32)
    b3 = np.asarray(inputs["b3"], np.float32)
    W4 = np.asarray(inputs["W4"], np.float32)
    b4 = np.asarray(inputs["b4"], np.float32)

    consts = dict(
        wet=np.ascontiguousarray(We.T).astype(NPBF),
        wqt=_pack_kt(Wq.T).astype(NPBF),
        wkt=_pack_kt(Wk.T).astype(NPBF),
        wvt=_pack_kt(Wv.T).astype(NPBF),
        bvrow=bv.reshape(1, H).astype(NPBF),
        w1t=_pack_kt(np.ascontiguousarray(W1[:, :H].T)),
        w1l=np.ascontiguousarray(W1[:, H]).reshape(1, H),
        w2t=_pack_kt(np.ascontiguousarray(W2.T)),
        w3t=_pack_kt(np.ascontiguousarray(W3.T)),
        w4t=np.ascontiguousarray(W4.reshape(256).reshape(2, 128).T),
        be2=_pack_bias(be_),
        bq2=_pack_bias(bq),
        bk2=_pack_bias(bk),
        b12=_pack_bias(b1),
        b22=_pack_bias(b2),
        b32=_pack_bias(b3),
        ident=np.eye(128, dtype=np.float32).astype(NPBF),
        ones=np.ones((1, 128), dtype=np.float32).astype(NPBF),
    )
    for k in ("w1t", "w1l", "w2t"):
        consts[k] = consts[k].astype(NPBF)
    for k in ("w3t", "w4t"):
        consts[k] = consts[k].astype(np.float32)

    xt = np.ascontiguousarray(x.transpose(0, 2, 1)).astype(NPBF)
    in_maps = []
    for c in range(NCORES):
        sl = slice(c * BE, (c + 1) * BE)
        m = dict(consts)
        m["xt"] = xt[sl]
        m["mask"] = np.ascontiguousarray(mask[sl])
        m["action"] = np.ascontiguousarray(
            action[sl].reshape(1, BE * N)
        ).astype(NPBF)
        in_maps.append(m)

    nc = _get_program()
    res = run_bass_kernel_spmd(nc, in_maps, list(range(NCORES)))
    _CACHE["last_res"] = res

    aw_parts = []
    val_parts = []
    for c in range(NCORES):
        aw_parts.append(res.results[c]["a_w"])
        val_parts.append(res.results[c]["value"].reshape(BE, N, 1))
    a_w = np.concatenate(aw_parts, axis=0)
    value = np.concatenate(val_parts, axis=0) + b4.reshape(1, 1, 1)
    return value.astype(np.float32), a_w.astype(np.float32)


# revision 25
# speedup vs baseline: 1.3767x; 1.1796x over previous
"""Trainium2 Bass kernel for the DGN-critic GNN message-passing module.

Contract: kernel(**inputs) takes the FULL unsharded inputs (as produced by
setup_inputs) and returns (value, a_w) matching reference().

Strategy: pure data parallel over B=1024 across 8 NeuronCores (128 batch
elems per core).  Within a core, batch elements are processed in groups of
4 so dense matmuls stream 512-wide.  Activations are kept feature-major
([feat, token]) so weights act as the stationary matmul operand and bias+
relu fold into the PSUM->SBUF copy.  Encoder/attention run in bf16 (fp32
accumulation); the critic MLP runs in float32r to keep the tiny `value`
output accurate.  Masking/softmax math is exact fp32.
"""

import numpy as np
import ml_dtypes

import concourse.bass as bass
import concourse.mybir as mybir
import concourse.tile as tile
from concourse.bass_utils import run_bass_kernel_spmd

F32 = mybir.dt.float32
F32R = mybir.dt.float32r
BF16 = mybir.dt.bfloat16
NPBF = ml_dtypes.bfloat16

NCORES = 8
B, N, OBS, H = 1024, 128, 64, 256
BE = B // NCORES          # batch elems per core
G = 4                     # elems per group (512-wide free dim)
NG = BE // G
BIG = 9e15
ESHIFT = -12.0            # constant softmax shift (scores observed <= ~10)


def split_multi_waits(nc):
    """Walrus in this container accepts at most ONE sync wait per
    instruction.  Hoist extra waits onto same-engine NOPs placed just
    before the instruction."""
    main_ctx = nc.cur_bb
    main_bb = main_ctx.bb
    for bbname, bbctx in list(nc.bb_map.items()):
        bb = bbctx.bb if hasattr(bbctx, "bb") else bbctx
        insts = list(bb.instructions)
        if not any(
            i.sync_info and i.sync_info.on_wait and len(i.sync_info.on_wait) > 1
            for i in insts
        ):
            continue
        new_list = []
        for inst in insts:
            si = inst.sync_info
            waits = list(si.on_wait) if si and si.on_wait else []
            if len(waits) > 1:
                for w in waits[:-1]:
                    nop = nc.engines[inst.engine].nop(nofuse=True)
                    nop_inst = (
                        nc.inst_map[nop.ins] if isinstance(nop.ins, str) else nop.ins
                    )
                    # nop() appended itself to the current bb; remove it.
                    lst = main_bb.instructions
                    lst = [i for i in lst if i.name != nop_inst.name]
                    main_bb.instructions = lst
                    nop_inst.sync_info = mybir.SyncInfo(on_wait=[w], on_update=[])
                    new_list.append(nop_inst)
                inst.sync_info = mybir.SyncInfo(
                    on_wait=[waits[-1]], on_update=list(si.on_update or [])
                )
            new_list.append(inst)
        bb.instructions = new_list


def build_program():
    nc = bass.Bass()

    # ---- DRAM I/O (per-core shard) ----
    xt_d = nc.dram_tensor("xt", [BE, OBS, N], BF16, kind="ExternalInput")
    mask_d = nc.dram_tensor("mask", [BE, N, N], F32, kind="ExternalInput")
    act_d = nc.dram_tensor("action", [1, BE * N], BF16, kind="ExternalInput")
    # weights, host-prepacked
    wet_d = nc.dram_tensor("wet", [OBS, H], BF16, kind="ExternalInput")
    wqt_d = nc.dram_tensor("wqt", [128, 2, H], BF16, kind="ExternalInput")
    wkt_d = nc.dram_tensor("wkt", [128, 2, H], BF16, kind="ExternalInput")
    wvt_d = nc.dram_tensor("wvt", [128, 2, H], BF16, kind="ExternalInput")
    bvrow_d = nc.dram_tensor("bvrow", [1, H], BF16, kind="ExternalInput")
    w1t_d = nc.dram_tensor("w1t", [128, 2, H], BF16, kind="ExternalInput")
    w1l_d = nc.dram_tensor("w1l", [1, H], BF16, kind="ExternalInput")
    w2t_d = nc.dram_tensor("w2t", [128, 2, H], BF16, kind="ExternalInput")
    w3t_d = nc.dram_tensor("w3t", [128, 2, H], F32R, kind="ExternalInput")
    w4t_d = nc.dram_tensor("w4t", [128, 2], F32R, kind="ExternalInput")
    be_d = nc.dram_tensor("be2", [128, 2], F32, kind="ExternalInput")
    bq_d = nc.dram_tensor("bq2", [128, 2], F32, kind="ExternalInput")
    bk_d = nc.dram_tensor("bk2", [128, 2], F32, kind="ExternalInput")
    b1_d = nc.dram_tensor("b12", [128, 2], F32, kind="ExternalInput")
    b2_d = nc.dram_tensor("b22", [128, 2], F32, kind="ExternalInput")
    b3_d = nc.dram_tensor("b32", [128, 2], F32, kind="ExternalInput")
    ident_d = nc.dram_tensor("ident", [128, 128], BF16, kind="ExternalInput")
    ones_d = nc.dram_tensor("ones", [1, 128], BF16, kind="ExternalInput")

    aw_d = nc.dram_tensor("a_w", [BE, N, N], F32, kind="ExternalOutput")
    val_d = nc.dram_tensor("value", [1, BE * N], F32, kind="ExternalOutput")

    b4_f = None  # bias b4 is passed via host fold (scalar) - set in kernel()

    Relu = mybir.ActivationFunctionType.Relu
    Copy = mybir.ActivationFunctionType.Copy
    Exp = mybir.ActivationFunctionType.Exp
    Alu = mybir.AluOpType

    from contextlib import ExitStack

    with tile.TileContext(nc) as tc, ExitStack() as ctx:
        const = ctx.enter_context(tc.tile_pool(name="const", bufs=1))
        io = ctx.enter_context(tc.tile_pool(name="io", bufs=4))
        wk = ctx.enter_context(tc.tile_pool(name="wk", bufs=3))
        awp = ctx.enter_context(tc.tile_pool(name="awp", bufs=3))
        pmm = ctx.enter_context(tc.tile_pool(name="pmm", bufs=5, space="PSUM"))
        psm = ctx.enter_context(tc.tile_pool(name="psm", bufs=3, space="PSUM"))

        # ---- preload constants ----
        def load_const(d, shape, dtype, name):
            t = const.tile(shape, dtype, tag=name)
            nc.sync.dma_start(t[:], d[:])
            return t

        wet = load_const(wet_d, [OBS, H], BF16, "wet")
        wqt = load_const(wqt_d, [128, 2, H], BF16, "wqt")
        wkt = load_const(wkt_d, [128, 2, H], BF16, "wkt")
        wvt = load_const(wvt_d, [128, 2, H], BF16, "wvt")
        bvrow = load_const(bvrow_d, [1, H], BF16, "bvrow")
        w1t = load_const(w1t_d, [128, 2, H], BF16, "w1t")
        w1l = load_const(w1l_d, [1, H], BF16, "w1l")
        w2t = load_const(w2t_d, [128, 2, H], BF16, "w2t")
        w3t = load_const(w3t_d, [128, 2, H], F32R, "w3t")
        w4t = load_const(w4t_d, [128, 2], F32R, "w4t")
        be2 = load_const(be_d, [128, 2], F32, "be2")
        bq2 = load_const(bq_d, [128, 2], F32, "bq2")
        bk2 = load_const(bk_d, [128, 2], F32, "bk2")
        b12 = load_const(b1_d, [128, 2], F32, "b12")
        b22 = load_const(b2_d, [128, 2], F32, "b22")
        b32 = load_const(b3_d, [128, 2], F32, "b32")
        ident = load_const(ident_d, [128, 128], BF16, "ident")
        ones1 = load_const(ones_d, [1, 128], BF16, "ones1")
        actt = load_const(act_d, [1, BE * N], BF16, "actt")
        eshift = const.tile([128, 1], F32, tag="eshift")
        nc.vector.memset(eshift[:], ESHIFT)

        def seg_load(g):
            g4 = g * G
            st = {}
            xT = io.tile([OBS, G, 128], BF16, tag="xT")
            nc.sync.dma_start(
                out=xT[:], in_=xt_d[g4 : g4 + G].rearrange("e d n -> d e n")
            )
            mask_g = io.tile([128, G, N], F32, tag="mask_g")
            nc.sync.dma_start(
                out=mask_g[:], in_=mask_d[g4 : g4 + G].rearrange("e n m -> n e m")
            )
            st["mask_g"] = mask_g
            st["xT"] = xT
            return st

        def seg_h1(g, st):
            xT = st["xT"]
            h1 = wk.tile([128, 2, 512], BF16, tag="h1")
            for m in range(2):
                ps = pmm.tile([128, 512], F32, tag="mm")
                nc.tensor.matmul(
                    ps[:],
                    wet[:, m * 128 : (m + 1) * 128],
                    xT.rearrange("d e n -> d (e n)"),
                    start=True,
                    stop=True,
                )
                nc.scalar.activation(
                    h1[:, m, :], ps[:], Relu, bias=be2[:, m : m + 1], scale=1.0
                )
            st["h1"] = h1

        def _qk_layer(h1, wt, bias2, tag):
            o = wk.tile([128, 2, 512], BF16, tag=tag)
            for m in range(2):
                ps = pmm.tile([128, 512], F32, tag="mm")
                for kt in range(2):
                    nc.tensor.matmul(
                        ps[:],
                        wt[:, kt, m * 128 : (m + 1) * 128],
                        h1[:, kt, :],
                        start=(kt == 0),
                        stop=(kt == 1),
                    )
                nc.scalar.activation(
                    o[:, m, :], ps[:], Relu, bias=bias2[:, m : m + 1], scale=1.0
                )
            return o

        def seg_q(g, st):
            st["qT"] = _qk_layer(st["h1"], wqt, bq2, "qT")

        def seg_k(g, st):
            st["kT"] = _qk_layer(st["h1"], wkt, bk2, "kT")

        def seg_v(g, st):
            h1 = st["h1"]
            v_bf = wk.tile([128, G, H], BF16, tag="v_bf")
            for e in range(G):
                ps = psm.tile([128, H], F32, tag="sm")
                for kt in range(2):
                    nc.tensor.matmul(
                        ps[:],
                        h1[:, kt, e * 128 : (e + 1) * 128],
                        wvt[:, kt, :],
                        start=(kt == 0),
                        stop=False,
                    )
                nc.tensor.matmul(ps[:], ones1[:], bvrow[:], start=False, stop=True)
                nc.vector.tensor_scalar(
                    out=v_bf[:, e, :], in0=ps[:], scalar1=0.0, scalar2=None,
                    op0=Alu.max,
                )
            st["v_bf"] = v_bf

        def seg_scores(g, st):
            g4 = g * G
            qT, kT, mask_g = st["qT"], st["kT"], st["mask_g"]
            nb = wk.tile([128, G * N], F32, tag="nb")
            nc.gpsimd.tensor_scalar(
                out=nb[:], in0=mask_g.rearrange("n e m -> n (e m)"),
                scalar1=BIG, scalar2=-BIG, op0=Alu.mult, op1=Alu.add,
            )
            t_g = wk.tile([128, G, N], F32, tag="t_g")
            for e in range(G):
                sps = psm.tile([128, N], F32, tag="sm")
                for kt in range(2):
                    nc.tensor.matmul(
                        sps[:],
                        qT[:, kt, e * 128 : (e + 1) * 128],
                        kT[:, kt, e * 128 : (e + 1) * 128],
                        start=(kt == 0),
                        stop=(kt == 1),
                    )
                nc.vector.scalar_tensor_tensor(
                    out=t_g[:, e, :], in0=sps[:], scalar=0.0, in1=mask_g[:, e, :],
                    op0=Alu.max, op1=Alu.mult,
                )
            a_w = awp.tile([128, G, N], F32, tag="a_w")
            nc.gpsimd.tensor_add(
                a_w.rearrange("n e m -> n (e m)"),
                t_g.rearrange("n e m -> n (e m)"),
                nb[:],
            )
            nc.sync.dma_start(
                out=aw_d[g4 : g4 + G].rearrange("e n m -> n e m"), in_=a_w[:]
            )
            E_g = wk.tile([128, G, N], BF16, tag="E_g")
            nc.scalar.activation(
                E_g.rearrange("n e m -> n (e m)"),
                a_w.rearrange("n e m -> n (e m)"),
                Exp, bias=eshift[:], scale=1.0,
            )
            ssum = wk.tile([128, G], F32, tag="ssum")
            nc.vector.tensor_reduce(
                ssum[:], E_g[:], axis=mybir.AxisListType.X, op=Alu.add
            )
            rinv = wk.tile([128, G], F32, tag="rinv")
            nc.vector.reciprocal(rinv[:], ssum[:])
            att = wk.tile([128, G, N], BF16, tag="att")
            for e in range(G):
                nc.vector.tensor_scalar(
                    out=att[:, e, :], in0=E_g[:, e, :],
                    scalar1=rinv[:, e : e + 1], scalar2=None, op0=Alu.mult,
                )
            st["att"] = att

        def seg_attT(g, st):
            aT_ps = pmm.tile([128, G * 128], BF16, tag="mm")
            att = st["att"]
            for e in range(G):
                nc.tensor.transpose(
                    aT_ps[:, e * 128 : (e + 1) * 128], att[:, e, :], ident[:]
                )
            aT = wk.tile([128, G * 128], BF16, tag="aT")
            nc.scalar.activation(aT[:], aT_ps[:], Copy)
            st["aT"] = aT

        def seg_h2(g, st):
            v_bf, aT = st["v_bf"], st["aT"]
            h2 = wk.tile([128, 2, 512], BF16, tag="h2")
            for m in range(2):
                ps = pmm.tile([128, 512], F32, tag="mm")
                for e in range(G):
                    nc.tensor.matmul(
                        ps[:, e * 128 : (e + 1) * 128],
                        v_bf[:, e, m * 128 : (m + 1) * 128],
                        aT[:, e * 128 : (e + 1) * 128],
                        start=True,
                        stop=True,
                    )
                nc.scalar.activation(h2[:, m, :], ps[:], Copy)
            st["h2"] = h2

        def _critic_layer(g, inp, wt, bias2, tag, extra=None, act_copies=(),
                          odt=F32R):
            o = wk.tile([128, 2, 512], odt, tag=tag)
            for m in range(2):
                ps = pmm.tile([128, 512], F32, tag="mm")
                nfin = 2 if extra is None else 3
                idx = 0
                for kt in range(2):
                    idx += 1
                    nc.tensor.matmul(
                        ps[:],
                        wt[:, kt, m * 128 : (m + 1) * 128],
                        inp[:, kt, :],
                        start=(kt == 0),
                        stop=(idx == nfin),
                    )
                if extra is not None:
                    w1l_t, act_slab = extra
                    idx += 1
                    nc.tensor.matmul(
                        ps[:],
                        w1l_t[:, m * 128 : (m + 1) * 128],
                        act_slab,
                        start=False,
                        stop=(idx == nfin),
                    )
                if m in act_copies:
                    nc.scalar.activation(
                        o[:, m, :], ps[:], Relu,
                        bias=bias2[:, m : m + 1], scale=1.0,
                    )
                else:
                    nc.vector.tensor_scalar(
                        out=o[:, m, :], in0=ps[:],
                        scalar1=bias2[:, m : m + 1], scalar2=0.0,
                        op0=Alu.add, op1=Alu.max,
                    )
            return o

        def seg_c2(g, st):
            g4 = g * G
            act_slab = actt[:, g4 * N : (g4 + G) * N]
            st["c2"] = _critic_layer(
                g, st["h2"], w1t, b12, "c2", extra=(w1l, act_slab), odt=BF16
            )

        def seg_c3(g, st):
            st["c3"] = _critic_layer(g, st["c2"], w2t, b22, "c3")

        def seg_c4(g, st):
            st["c4"] = _critic_layer(g, st["c3"], w3t, b32, "c4", act_copies=(0,))

        def seg_value(g, st):
            c4 = st["c4"]
            vps = psm.tile([1, 512], F32, tag="sm")
            for kt in range(2):
                nc.tensor.matmul(
                    vps[:],
                    w4t[:, kt : kt + 1],
                    c4[:, kt, :],
                    start=(kt == 0),
                    stop=(kt == 1),
                )
            val_sb = wk.tile([1, 512], F32, tag="val_sb")
            nc.scalar.activation(val_sb[:], vps[:], Copy)
            nc.sync.dma_start(
                out=val_d[:, g * 512 : (g + 1) * 512], in_=val_sb[:]
            )

        # 3-deep software pipeline: per outer step, interleave segments of
        # groups a=t (fresh), b=t-1 (attention tail), c=t-2 (critic tail)
        # so every cross-engine handoff is covered by PE work from another
        # group.
        state = {0: seg_load(0)}
        for t in range(NG + 2):
            a, bq, cq = t, t - 1, t - 2
            ina = a < NG
            inb = 0 <= bq < NG
            inc_ = 0 <= cq
            if a + 1 < NG:
                state[a + 1] = seg_load(a + 1)
            if ina:
                seg_h1(a, state[a])
            if inc_:
                seg_c4(cq, state[cq])
            if inb:
                seg_attT(bq, state[bq])
            if ina:
                seg_q(a, state[a])
            if inb:
                seg_h2(bq, state[bq])
            if ina:
                seg_k(a, state[a])
            if inc_:
                seg_value(cq, state.pop(cq))
            if inb:
                seg_c2(bq, state[bq])
            if ina:
                seg_v(a, state[a])
            if inb:
                seg_c3(bq, state[bq])
            if ina:
                seg_scores(a, state[a])

    split_multi_waits(nc)
    return nc


_CACHE = {}


def _get_program():
    if "nc" not in _CACHE:
        _CACHE["nc"] = build_program()
    return _CACHE["nc"]


def _pack_kt(wT):
    """[256, out] -> [128, 2, out] with kt-major partition packing."""
    out = wT.shape[1]
    return np.ascontiguousarray(
        wT.reshape(2, 128, out).transpose(1, 0, 2)
    )


def _pack_bias(b):
    return np.ascontiguousarray(b.reshape(2, 128).T)


def kernel(**inputs):
    x = np.asarray(inputs["x"], np.float32)
    mask = np.asarray(inputs["mask"], np.float32)
    action = np.asarray(inputs["action"], np.float32)
    We = np.asarray(inputs["We"], np.float32)
    be_ = np.asarray(inputs["be"], np.float32)
    Wv = np.asarray(inputs["Wv"], np.float32)
    bv = np.asarray(inputs["bv"], np.float32)
    Wk = np.asarray(inputs["Wk"], np.float32)
    bk = np.asarray(inputs["bk"], np.float32)
    Wq = np.asarray(inputs["Wq"], np.float32)
    bq = np.asarray(inputs["bq"], np.float32)
    W1 = np.asarray(inputs["W1"], np.float32)
    b1 = np.asarray(inputs["b1"], np.float32)
    W2 = np.asarray(inputs["W2"], np.float32)
    b2 = np.asarray(inputs["b2"], np.float32)
    W3 = np.asarray(inputs["W3"], np.float32)
    b3 = np.asarray(inputs["b3"], np.float32)
    W4 = np.asarray(inputs["W4"], np.float32)
    b4 = np.asarray(inputs["b4"], np.float32)

    consts = dict(
        wet=np.ascontiguousarray(We.T).astype(NPBF),
        wqt=_pack_kt(Wq.T).astype(NPBF),
        wkt=_pack_kt(Wk.T).astype(NPBF),
        wvt=_pack_kt(Wv.T).astype(NPBF),
        bvrow=bv.reshape(1, H).astype(NPBF),
        w1t=_pack_kt(np.ascontiguousarray(W1[:, :H].T)),
        w1l=np.ascontiguousarray(W1[:, H]).reshape(1, H),
        w2t=_pack_kt(np.ascontiguousarray(W2.T)),
        w3t=_pack_kt(np.ascontiguousarray(W3.T)),
        w4t=np.ascontiguousarray(W4.reshape(256).reshape(2, 128).T),
        be2=_pack_bias(be_),
        bq2=_pack_bias(bq),
        bk2=_pack_bias(bk),
        b12=_pack_bias(b1),
        b22=_pack_bias(b2),
        b32=_pack_bias(b3),
        ident=np.eye(128, dtype=np.float32).astype(NPBF),
        ones=np.ones((1, 128), dtype=np.float32).astype(NPBF),
    )
    for k in ("w1t", "w1l", "w2t"):
        consts[k] = consts[k].astype(NPBF)
    for k in ("w3t", "w4t"):
        consts[k] = consts[k].astype(np.float32)

    xt = np.ascontiguousarray(x.transpose(0, 2, 1)).astype(NPBF)
    in_maps = []
    for c in range(NCORES):
        sl = slice(c * BE, (c + 1) * BE)
        m = dict(consts)
        m["xt"] = xt[sl]
        m["mask"] = np.ascontiguousarray(mask[sl])
        m["action"] = np.ascontiguousarray(
            action[sl].reshape(1, BE * N)
        ).astype(NPBF)
        in_maps.append(m)

    nc = _get_program()
    res = run_bass_kernel_spmd(nc, in_maps, list(range(NCORES)))
    _CACHE["last_res"] = res

    aw_parts = []
    val_parts = []
    for c in range(NCORES):
        aw_parts.append(res.results[c]["a_w"])
        val_parts.append(res.results[c]["value"].reshape(BE, N, 1))
    a_w = np.concatenate(aw_parts, axis=0)
    value = np.concatenate(val_parts, axis=0) + b4.reshape(1, 1, 1)
    return value.astype(np.float32), a_w.astype(np.float32)


# revision 26
# speedup vs baseline: 1.3924x; 1.0114x over previous
"""Trainium2 Bass kernel for the DGN-critic GNN message-passing module.

Contract: kernel(**inputs) takes the FULL unsharded inputs (as produced by
setup_inputs) and returns (value, a_w) matching reference().

Strategy: pure data parallel over B=1024 across 8 NeuronCores (128 batch
elems per core).  Within a core, batch elements are processed in groups of
4 so dense matmuls stream 512-wide.  Activations are kept feature-major
([feat, token]) so weights act as the stationary matmul operand and bias+
relu fold into the PSUM->SBUF copy.  Encoder/attention run in bf16 (fp32
accumulation); the critic MLP runs in float32r to keep the tiny `value`
output accurate.  Masking/softmax math is exact fp32.
"""

import numpy as np
import ml_dtypes

import concourse.bass as bass
import concourse.mybir as mybir
import concourse.tile as tile
from concourse.bass_utils import run_bass_kernel_spmd

F32 = mybir.dt.float32
F32R = mybir.dt.float32r
BF16 = mybir.dt.bfloat16
NPBF = ml_dtypes.bfloat16

NCORES = 8
B, N, OBS, H = 1024, 128, 64, 256
BE = B // NCORES          # batch elems per core
G = 4                     # elems per group (512-wide free dim)
NG = BE // G
BIG = 9e15
ESHIFT = -12.0            # constant softmax shift (scores observed <= ~10)


def split_multi_waits(nc):
    """Walrus in this container accepts at most ONE sync wait per
    instruction.  Hoist extra waits onto same-engine NOPs placed just
    before the instruction."""
    main_ctx = nc.cur_bb
    main_bb = main_ctx.bb
    for bbname, bbctx in list(nc.bb_map.items()):
        bb = bbctx.bb if hasattr(bbctx, "bb") else bbctx
        insts = list(bb.instructions)
        if not any(
            i.sync_info and i.sync_info.on_wait and len(i.sync_info.on_wait) > 1
            for i in insts
        ):
            continue
        new_list = []
        for inst in insts:
            si = inst.sync_info
            waits = list(si.on_wait) if si and si.on_wait else []
            if len(waits) > 1:
                for w in waits[:-1]:
                    nop = nc.engines[inst.engine].nop(nofuse=True)
                    nop_inst = (
                        nc.inst_map[nop.ins] if isinstance(nop.ins, str) else nop.ins
                    )
                    # nop() appended itself to the current bb; remove it.
                    lst = main_bb.instructions
                    lst = [i for i in lst if i.name != nop_inst.name]
                    main_bb.instructions = lst
                    nop_inst.sync_info = mybir.SyncInfo(on_wait=[w], on_update=[])
                    new_list.append(nop_inst)
                inst.sync_info = mybir.SyncInfo(
                    on_wait=[waits[-1]], on_update=list(si.on_update or [])
                )
            new_list.append(inst)
        bb.instructions = new_list


def build_program():
    nc = bass.Bass()

    # ---- DRAM I/O (per-core shard) ----
    xt_d = nc.dram_tensor("xt", [BE, OBS, N], BF16, kind="ExternalInput")
    mask_d = nc.dram_tensor("mask", [BE, N, N], F32, kind="ExternalInput")
    act_d = nc.dram_tensor("action", [1, BE * N], BF16, kind="ExternalInput")
    # weights, host-prepacked
    wet_d = nc.dram_tensor("wet", [OBS, H], BF16, kind="ExternalInput")
    wqt_d = nc.dram_tensor("wqt", [128, 2, H], BF16, kind="ExternalInput")
    wkt_d = nc.dram_tensor("wkt", [128, 2, H], BF16, kind="ExternalInput")
    wvt_d = nc.dram_tensor("wvt", [128, 2, H], BF16, kind="ExternalInput")
    bvrow_d = nc.dram_tensor("bvrow", [1, H], BF16, kind="ExternalInput")
    w1t_d = nc.dram_tensor("w1t", [128, 2, H], BF16, kind="ExternalInput")
    w1l_d = nc.dram_tensor("w1l", [1, H], BF16, kind="ExternalInput")
    w2t_d = nc.dram_tensor("w2t", [128, 2, H], BF16, kind="ExternalInput")
    w3t_d = nc.dram_tensor("w3t", [128, 2, H], F32R, kind="ExternalInput")
    w4t_d = nc.dram_tensor("w4t", [128, 2], F32R, kind="ExternalInput")
    be_d = nc.dram_tensor("be2", [128, 2], F32, kind="ExternalInput")
    bq_d = nc.dram_tensor("bq2", [128, 2], F32, kind="ExternalInput")
    bk_d = nc.dram_tensor("bk2", [128, 2], F32, kind="ExternalInput")
    b1_d = nc.dram_tensor("b12", [128, 2], F32, kind="ExternalInput")
    b2_d = nc.dram_tensor("b22", [128, 2], F32, kind="ExternalInput")
    b3_d = nc.dram_tensor("b32", [128, 2], F32, kind="ExternalInput")
    ident_d = nc.dram_tensor("ident", [128, 128], BF16, kind="ExternalInput")
    ones_d = nc.dram_tensor("ones", [1, 128], BF16, kind="ExternalInput")

    aw_d = nc.dram_tensor("a_w", [BE, N, N], F32, kind="ExternalOutput")
    val_d = nc.dram_tensor("value", [1, BE * N], F32, kind="ExternalOutput")

    b4_f = None  # bias b4 is passed via host fold (scalar) - set in kernel()

    Relu = mybir.ActivationFunctionType.Relu
    Copy = mybir.ActivationFunctionType.Copy
    Exp = mybir.ActivationFunctionType.Exp
    Alu = mybir.AluOpType

    from contextlib import ExitStack

    with tile.TileContext(nc) as tc, ExitStack() as ctx:
        const = ctx.enter_context(tc.tile_pool(name="const", bufs=1))
        io = ctx.enter_context(tc.tile_pool(name="io", bufs=4))
        wk = ctx.enter_context(tc.tile_pool(name="wk", bufs=4))
        awp = ctx.enter_context(tc.tile_pool(name="awp", bufs=3))
        pmm = ctx.enter_context(tc.tile_pool(name="pmm", bufs=5, space="PSUM"))
        psm = ctx.enter_context(tc.tile_pool(name="psm", bufs=3, space="PSUM"))

        # ---- preload constants ----
        def load_const(d, shape, dtype, name):
            t = const.tile(shape, dtype, tag=name)
            nc.sync.dma_start(t[:], d[:])
            return t

        wet = load_const(wet_d, [OBS, H], BF16, "wet")
        be2 = load_const(be_d, [128, 2], F32, "be2")
        wqt = load_const(wqt_d, [128, 2, H], BF16, "wqt")
        wkt = load_const(wkt_d, [128, 2, H], BF16, "wkt")
        bq2 = load_const(bq_d, [128, 2], F32, "bq2")
        bk2 = load_const(bk_d, [128, 2], F32, "bk2")
        wvt = load_const(wvt_d, [128, 2, H], BF16, "wvt")
        bvrow = load_const(bvrow_d, [1, H], BF16, "bvrow")
        ones1 = load_const(ones_d, [1, 128], BF16, "ones1")
        ident = load_const(ident_d, [128, 128], BF16, "ident")
        eshift = const.tile([128, 1], F32, tag="eshift")
        nc.vector.memset(eshift[:], ESHIFT)
        # group-0 inputs load before the bulky critic weights so the first
        # h1/qk matmuls can start ~10us earlier.
        _g0_xT = io.tile([OBS, G, 128], BF16, tag="xT")
        nc.sync.dma_start(
            out=_g0_xT[:], in_=xt_d[0:G].rearrange("e d n -> d e n")
        )
        _g0_mask = io.tile([128, G, N], F32, tag="mask_g")
        nc.sync.dma_start(
            out=_g0_mask[:], in_=mask_d[0:G].rearrange("e n m -> n e m")
        )
        _g0_state = {"xT": _g0_xT, "mask_g": _g0_mask}
        # critic weights load later - first groups' encoder work can start.
        w1t = load_const(w1t_d, [128, 2, H], BF16, "w1t")
        w1l = load_const(w1l_d, [1, H], BF16, "w1l")
        w2t = load_const(w2t_d, [128, 2, H], BF16, "w2t")
        w3t = load_const(w3t_d, [128, 2, H], F32R, "w3t")
        w4t = load_const(w4t_d, [128, 2], F32R, "w4t")
        b12 = load_const(b1_d, [128, 2], F32, "b12")
        b22 = load_const(b2_d, [128, 2], F32, "b22")
        b32 = load_const(b3_d, [128, 2], F32, "b32")
        actt = load_const(act_d, [1, BE * N], BF16, "actt")

        def seg_load(g):
            g4 = g * G
            st = {}
            xT = io.tile([OBS, G, 128], BF16, tag="xT")
            nc.sync.dma_start(
                out=xT[:], in_=xt_d[g4 : g4 + G].rearrange("e d n -> d e n")
            )
            mask_g = io.tile([128, G, N], F32, tag="mask_g")
            nc.sync.dma_start(
                out=mask_g[:], in_=mask_d[g4 : g4 + G].rearrange("e n m -> n e m")
            )
            st["mask_g"] = mask_g
            st["xT"] = xT
            return st

        def seg_h1(g, st):
            xT = st["xT"]
            h1 = wk.tile([128, 2, 512], BF16, tag="h1")
            for m in range(2):
                ps = pmm.tile([128, 512], F32, tag="mm")
                nc.tensor.matmul(
                    ps[:],
                    wet[:, m * 128 : (m + 1) * 128],
                    xT.rearrange("d e n -> d (e n)"),
                    start=True,
                    stop=True,
                )
                nc.scalar.activation(
                    h1[:, m, :], ps[:], Relu, bias=be2[:, m : m + 1], scale=1.0
                )
            st["h1"] = h1

        def _qk_layer(h1, wt, bias2, tag):
            o = wk.tile([128, 2, 512], BF16, tag=tag)
            for m in range(2):
                ps = pmm.tile([128, 512], F32, tag="mm")
                for kt in range(2):
                    nc.tensor.matmul(
                        ps[:],
                        wt[:, kt, m * 128 : (m + 1) * 128],
                        h1[:, kt, :],
                        start=(kt == 0),
                        stop=(kt == 1),
                    )
                nc.scalar.activation(
                    o[:, m, :], ps[:], Relu, bias=bias2[:, m : m + 1], scale=1.0
                )
            return o

        def seg_q(g, st):
            st["qT"] = _qk_layer(st["h1"], wqt, bq2, "qT")

        def seg_k(g, st):
            st["kT"] = _qk_layer(st["h1"], wkt, bk2, "kT")

        def seg_v(g, st):
            h1 = st["h1"]
            v_bf = wk.tile([128, G, H], BF16, tag="v_bf")
            for e in range(G):
                ps = psm.tile([128, H], F32, tag="sm")
                for kt in range(2):
                    nc.tensor.matmul(
                        ps[:],
                        h1[:, kt, e * 128 : (e + 1) * 128],
                        wvt[:, kt, :],
                        start=(kt == 0),
                        stop=False,
                    )
                nc.tensor.matmul(ps[:], ones1[:], bvrow[:], start=False, stop=True)
                nc.vector.tensor_scalar(
                    out=v_bf[:, e, :], in0=ps[:], scalar1=0.0, scalar2=None,
                    op0=Alu.max,
                )
            st["v_bf"] = v_bf

        def seg_scores(g, st):
            g4 = g * G
            qT, kT, mask_g = st["qT"], st["kT"], st["mask_g"]
            nb = wk.tile([128, G * N], F32, tag="nb")
            nc.gpsimd.tensor_scalar(
                out=nb[:], in0=mask_g.rearrange("n e m -> n (e m)"),
                scalar1=BIG, scalar2=-BIG, op0=Alu.mult, op1=Alu.add,
            )
            t_g = wk.tile([128, G, N], F32, tag="t_g")
            for e in range(G):
                sps = psm.tile([128, N], F32, tag="sm")
                for kt in range(2):
                    nc.tensor.matmul(
                        sps[:],
                        qT[:, kt, e * 128 : (e + 1) * 128],
                        kT[:, kt, e * 128 : (e + 1) * 128],
                        start=(kt == 0),
                        stop=(kt == 1),
                    )
                nc.vector.scalar_tensor_tensor(
                    out=t_g[:, e, :], in0=sps[:], scalar=0.0, in1=mask_g[:, e, :],
                    op0=Alu.max, op1=Alu.mult,
                )
            a_w = awp.tile([128, G, N], F32, tag="a_w")
            nc.gpsimd.tensor_add(
                a_w.rearrange("n e m -> n (e m)"),
                t_g.rearrange("n e m -> n (e m)"),
                nb[:],
            )
            nc.sync.dma_start(
                out=aw_d[g4 : g4 + G].rearrange("e n m -> n e m"), in_=a_w[:]
            )
            E_g = wk.tile([128, G, N], BF16, tag="E_g")
            nc.scalar.activation(
                E_g.rearrange("n e m -> n (e m)"),
                a_w.rearrange("n e m -> n (e m)"),
                Exp, bias=eshift[:], scale=1.0,
            )
            ssum = wk.tile([128, G], F32, tag="ssum")
            nc.vector.tensor_reduce(
                ssum[:], E_g[:], axis=mybir.AxisListType.X, op=Alu.add
            )
            rinv = wk.tile([128, G], F32, tag="rinv")
            nc.vector.reciprocal(rinv[:], ssum[:])
            att = wk.tile([128, G, N], BF16, tag="att")
            for e in range(G):
                nc.vector.tensor_scalar(
                    out=att[:, e, :], in0=E_g[:, e, :],
                    scalar1=rinv[:, e : e + 1], scalar2=None, op0=Alu.mult,
                )
            st["att"] = att

        def seg_attT(g, st):
            aT_ps = pmm.tile([128, G * 128], BF16, tag="mm")
            att = st["att"]
            for e in range(G):
                nc.tensor.transpose(
                    aT_ps[:, e * 128 : (e + 1) * 128], att[:, e, :], ident[:]
                )
            aT = wk.tile([128, G * 128], BF16, tag="aT")
            nc.scalar.activation(aT[:], aT_ps[:], Copy)
            st["aT"] = aT

        def seg_h2(g, st):
            v_bf, aT = st["v_bf"], st["aT"]
            h2 = wk.tile([128, 2, 512], BF16, tag="h2")
            for m in range(2):
                ps = pmm.tile([128, 512], F32, tag="mm")
                for e in range(G):
                    nc.tensor.matmul(
                        ps[:, e * 128 : (e + 1) * 128],
                        v_bf[:, e, m * 128 : (m + 1) * 128],
                        aT[:, e * 128 : (e + 1) * 128],
                        start=True,
                        stop=True,
                    )
                nc.scalar.activation(h2[:, m, :], ps[:], Copy)
            st["h2"] = h2

        def _critic_layer(g, inp, wt, bias2, tag, extra=None, act_copies=(),
                          odt=F32R):
            o = wk.tile([128, 2, 512], odt, tag=tag)
            for m in range(2):
                ps = pmm.tile([128, 512], F32, tag="mm")
                nfin = 2 if extra is None else 3
                idx = 0
                for kt in range(2):
                    idx += 1
                    nc.tensor.matmul(
                        ps[:],
                        wt[:, kt, m * 128 : (m + 1) * 128],
                        inp[:, kt, :],
                        start=(kt == 0),
                        stop=(idx == nfin),
                    )
                if extra is not None:
                    w1l_t, act_slab = extra
                    idx += 1
                    nc.tensor.matmul(
                        ps[:],
                        w1l_t[:, m * 128 : (m + 1) * 128],
                        act_slab,
                        start=False,
                        stop=(idx == nfin),
                    )
                if m in act_copies:
                    nc.scalar.activation(
                        o[:, m, :], ps[:], Relu,
                        bias=bias2[:, m : m + 1], scale=1.0,
                    )
                else:
                    nc.vector.tensor_scalar(
                        out=o[:, m, :], in0=ps[:],
                        scalar1=bias2[:, m : m + 1], scalar2=0.0,
                        op0=Alu.add, op1=Alu.max,
                    )
            return o

        def seg_c2(g, st):
            g4 = g * G
            act_slab = actt[:, g4 * N : (g4 + G) * N]
            st["c2"] = _critic_layer(
                g, st["h2"], w1t, b12, "c2", extra=(w1l, act_slab), odt=BF16
            )

        def seg_c3(g, st):
            st["c3"] = _critic_layer(g, st["c2"], w2t, b22, "c3")

        def seg_c4(g, st):
            st["c4"] = _critic_layer(g, st["c3"], w3t, b32, "c4", act_copies=(0,))

        def seg_value(g, st):
            c4 = st["c4"]
            vps = psm.tile([1, 512], F32, tag="sm")
            for kt in range(2):
                nc.tensor.matmul(
                    vps[:],
                    w4t[:, kt : kt + 1],
                    c4[:, kt, :],
                    start=(kt == 0),
                    stop=(kt == 1),
                )
            val_sb = wk.tile([1, 512], F32, tag="val_sb")
            nc.scalar.activation(val_sb[:], vps[:], Copy)
            nc.sync.dma_start(
                out=val_d[:, g * 512 : (g + 1) * 512], in_=val_sb[:]
            )

        # 3-deep software pipeline: per outer step, interleave segments of
        # groups a=t (fresh), b=t-1 (attention tail), c=t-2 (critic tail)
        # so every cross-engine handoff is covered by PE work from another
        # group.
        state = {0: _g0_state}
        for t in range(NG + 2):
            a, bq, cq = t, t - 1, t - 2
            ina = a < NG
            inb = 0 <= bq < NG
            inc_ = 0 <= cq
            if a + 1 < NG:
                state[a + 1] = seg_load(a + 1)
            if ina:
                seg_h1(a, state[a])
            if inc_:
                seg_c4(cq, state[cq])
            if inb:
                seg_attT(bq, state[bq])
            if ina:
                seg_q(a, state[a])
            if inb:
                seg_h2(bq, state[bq])
            if ina:
                seg_k(a, state[a])
            if inc_:
                seg_value(cq, state.pop(cq))
            if inb:
                seg_c2(bq, state[bq])
            if ina:
                seg_v(a, state[a])
            if inb:
                seg_c3(bq, state[bq])
            if ina:
                seg_scores(a, state[a])

    split_multi_waits(nc)
    return nc


_CACHE = {}


def _get_program():
    if "nc" not in _CACHE:
        _CACHE["nc"] = build_program()
    return _CACHE["nc"]


def _pack_kt(wT):
    """[256, out] -> [128, 2, out] with kt-major partition packing."""
    out = wT.shape[1]
    return np.ascontiguousarray(
        wT.reshape(2, 128, out).transpose(1, 0, 2)
    )


def _pack_bias(b):
    return np.ascontiguousarray(b.reshape(2, 128).T)


def kernel(**inputs):
    x = np.asarray(inputs["x"], np.float32)
    mask = np.asarray(inputs["mask"], np.float32)
    action = np.asarray(inputs["action"], np.float32)
    We = np.asarray(inputs["We"], np.float32)
    be_ = np.asarray(inputs["be"], np.float32)
    Wv = np.asarray(inputs["Wv"], np.float32)
    bv = np.asarray(inputs["bv"], np.float32)
    Wk = np.asarray(inputs["Wk"], np.float32)
    bk = np.asarray(inputs["bk"], np.float32)
    Wq = np.asarray(inputs["Wq"], np.float32)
    bq = np.asarray(inputs["bq"], np.float32)
    W1 = np.asarray(inputs["W1"], np.float32)
    b1 = np.asarray(inputs["b1"], np.float32)
    W2 = np.asarray(inputs["W2"], np.float32)
    b2 = np.asarray(inputs["b2"], np.float32)
    W3 = np.asarray(inputs["W3"], np.float32)
    b3 = np.asarray(inputs["b3"], np.float32)
    W4 = np.asarray(inputs["W4"], np.float32)
    b4 = np.asarray(inputs["b4"], np.float32)

    consts = dict(
        wet=np.ascontiguousarray(We.T).astype(NPBF),
        wqt=_pack_kt(Wq.T).astype(NPBF),
        wkt=_pack_kt(Wk.T).astype(NPBF),
        wvt=_pack_kt(Wv.T).astype(NPBF),
        bvrow=bv.reshape(1, H).astype(NPBF),
        w1t=_pack_kt(np.ascontiguousarray(W1[:, :H].T)),
        w1l=np.ascontiguousarray(W1[:, H]).reshape(1, H),
        w2t=_pack_kt(np.ascontiguousarray(W2.T)),
        w3t=_pack_kt(np.ascontiguousarray(W3.T)),
        w4t=np.ascontiguousarray(W4.reshape(256).reshape(2, 128).T),
        be2=_pack_bias(be_),
        bq2=_pack_bias(bq),
        bk2=_pack_bias(bk),
        b12=_pack_bias(b1),
        b22=_pack_bias(b2),
        b32=_pack_bias(b3),
        ident=np.eye(128, dtype=np.float32).astype(NPBF),
        ones=np.ones((1, 128), dtype=np.float32).astype(NPBF),
    )
    for k in ("w1t", "w1l", "w2t"):
        consts[k] = consts[k].astype(NPBF)
    for k in ("w3t", "w4t"):
        consts[k] = consts[k].astype(np.float32)

    xt = np.ascontiguousarray(x.transpose(0, 2, 1)).astype(NPBF)
    in_maps = []
    for c in range(NCORES):
        sl = slice(c * BE, (c + 1) * BE)
        m = dict(consts)
        m["xt"] = xt[sl]
        m["mask"] = np.ascontiguousarray(mask[sl])
        m["action"] = np.ascontiguousarray(
            action[sl].reshape(1, BE * N)
        ).astype(NPBF)
        in_maps.append(m)

    nc = _get_program()
    res = run_bass_kernel_spmd(nc, in_maps, list(range(NCORES)))
    _CACHE["last_res"] = res

    aw_parts = []
    val_parts = []
    for c in range(NCORES):
        aw_parts.append(res.results[c]["a_w"])
        val_parts.append(res.results[c]["value"].reshape(BE, N, 1))
    a_w = np.concatenate(aw_parts, axis=0)
    value = np.concatenate(val_parts, axis=0) + b4.reshape(1, 1, 1)
    return value.astype(np.float32), a_w.astype(np.float32)


# revision 27
# speedup vs baseline: 1.3939x; 1.0011x over previous
"""Trainium2 Bass kernel for the DGN-critic GNN message-passing module.

Contract: kernel(**inputs) takes the FULL unsharded inputs (as produced by
setup_inputs) and returns (value, a_w) matching reference().

Strategy: pure data parallel over B=1024 across 8 NeuronCores (128 batch
elems per core).  Within a core, batch elements are processed in groups of
4 so dense matmuls stream 512-wide.  Activations are kept feature-major
([feat, token]) so weights act as the stationary matmul operand and bias+
relu fold into the PSUM->SBUF copy.  Encoder/attention run in bf16 (fp32
accumulation); the critic MLP runs in float32r to keep the tiny `value`
output accurate.  Masking/softmax math is exact fp32.
"""

import numpy as np
import ml_dtypes

import concourse.bass as bass
import concourse.mybir as mybir
import concourse.tile as tile
from concourse.bass_utils import run_bass_kernel_spmd

F32 = mybir.dt.float32
F32R = mybir.dt.float32r
BF16 = mybir.dt.bfloat16
NPBF = ml_dtypes.bfloat16

NCORES = 8
B, N, OBS, H = 1024, 128, 64, 256
BE = B // NCORES          # batch elems per core
G = 4                     # elems per group (512-wide free dim)
NG = BE // G
BIG = 9e15
ESHIFT = -12.0            # constant softmax shift (scores observed <= ~10)


def split_multi_waits(nc):
    """Walrus in this container accepts at most ONE sync wait per
    instruction.  Hoist extra waits onto same-engine NOPs placed just
    before the instruction."""
    main_ctx = nc.cur_bb
    main_bb = main_ctx.bb
    for bbname, bbctx in list(nc.bb_map.items()):
        bb = bbctx.bb if hasattr(bbctx, "bb") else bbctx
        insts = list(bb.instructions)
        if not any(
            i.sync_info and i.sync_info.on_wait and len(i.sync_info.on_wait) > 1
            for i in insts
        ):
            continue
        new_list = []
        for inst in insts:
            si = inst.sync_info
            waits = list(si.on_wait) if si and si.on_wait else []
            if len(waits) > 1:
                for w in waits[:-1]:
                    nop = nc.engines[inst.engine].nop(nofuse=True)
                    nop_inst = (
                        nc.inst_map[nop.ins] if isinstance(nop.ins, str) else nop.ins
                    )
                    # nop() appended itself to the current bb; remove it.
                    lst = main_bb.instructions
                    lst = [i for i in lst if i.name != nop_inst.name]
                    main_bb.instructions = lst
                    nop_inst.sync_info = mybir.SyncInfo(on_wait=[w], on_update=[])
                    new_list.append(nop_inst)
                inst.sync_info = mybir.SyncInfo(
                    on_wait=[waits[-1]], on_update=list(si.on_update or [])
                )
            new_list.append(inst)
        bb.instructions = new_list


def build_program():
    nc = bass.Bass()

    # ---- DRAM I/O (per-core shard) ----
    xt_d = nc.dram_tensor("xt", [BE, OBS, N], BF16, kind="ExternalInput")
    mask_d = nc.dram_tensor("mask", [BE, N, N], F32, kind="ExternalInput")
    act_d = nc.dram_tensor("action", [1, BE * N], BF16, kind="ExternalInput")
    # weights, host-prepacked
    wet_d = nc.dram_tensor("wet", [OBS, H], BF16, kind="ExternalInput")
    wqt_d = nc.dram_tensor("wqt", [128, 2, H], BF16, kind="ExternalInput")
    wkt_d = nc.dram_tensor("wkt", [128, 2, H], BF16, kind="ExternalInput")
    wvt_d = nc.dram_tensor("wvt", [128, 2, H], BF16, kind="ExternalInput")
    bvrow_d = nc.dram_tensor("bvrow", [1, H], BF16, kind="ExternalInput")
    w1t_d = nc.dram_tensor("w1t", [128, 2, H], BF16, kind="ExternalInput")
    w1l_d = nc.dram_tensor("w1l", [1, H], BF16, kind="ExternalInput")
    w2t_d = nc.dram_tensor("w2t", [128, 2, H], BF16, kind="ExternalInput")
    w3t_d = nc.dram_tensor("w3t", [128, 2, H], F32R, kind="ExternalInput")
    w4t_d = nc.dram_tensor("w4t", [128, 2], F32R, kind="ExternalInput")
    be_d = nc.dram_tensor("be2", [128, 2], F32, kind="ExternalInput")
    bq_d = nc.dram_tensor("bq2", [128, 2], F32, kind="ExternalInput")
    bk_d = nc.dram_tensor("bk2", [128, 2], F32, kind="ExternalInput")
    b1_d = nc.dram_tensor("b12", [128, 2], F32, kind="ExternalInput")
    b2_d = nc.dram_tensor("b22", [128, 2], F32, kind="ExternalInput")
    b3_d = nc.dram_tensor("b32", [128, 2], F32, kind="ExternalInput")
    ident_d = nc.dram_tensor("ident", [128, 128], BF16, kind="ExternalInput")
    ones_d = nc.dram_tensor("ones", [1, 128], BF16, kind="ExternalInput")

    aw_d = nc.dram_tensor("a_w", [BE, N, N], F32, kind="ExternalOutput")
    val_d = nc.dram_tensor("value", [1, BE * N], F32, kind="ExternalOutput")

    b4_f = None  # bias b4 is passed via host fold (scalar) - set in kernel()

    Relu = mybir.ActivationFunctionType.Relu
    Copy = mybir.ActivationFunctionType.Copy
    Exp = mybir.ActivationFunctionType.Exp
    Alu = mybir.AluOpType

    from contextlib import ExitStack

    with tile.TileContext(nc) as tc, ExitStack() as ctx:
        const = ctx.enter_context(tc.tile_pool(name="const", bufs=1))
        io = ctx.enter_context(tc.tile_pool(name="io", bufs=4))
        wk = ctx.enter_context(tc.tile_pool(name="wk", bufs=4))
        awp = ctx.enter_context(tc.tile_pool(name="awp", bufs=3))
        pmm = ctx.enter_context(tc.tile_pool(name="pmm", bufs=5, space="PSUM"))
        psm = ctx.enter_context(tc.tile_pool(name="psm", bufs=3, space="PSUM"))

        # ---- preload constants ----
        def load_const(d, shape, dtype, name):
            t = const.tile(shape, dtype, tag=name)
            nc.sync.dma_start(t[:], d[:])
            return t

        wet = load_const(wet_d, [OBS, H], BF16, "wet")
        be2 = load_const(be_d, [128, 2], F32, "be2")
        wqt = load_const(wqt_d, [128, 2, H], BF16, "wqt")
        wkt = load_const(wkt_d, [128, 2, H], BF16, "wkt")
        bq2 = load_const(bq_d, [128, 2], F32, "bq2")
        bk2 = load_const(bk_d, [128, 2], F32, "bk2")
        wvt = load_const(wvt_d, [128, 2, H], BF16, "wvt")
        bvrow = load_const(bvrow_d, [1, H], BF16, "bvrow")
        ones1 = load_const(ones_d, [1, 128], BF16, "ones1")
        ident = load_const(ident_d, [128, 128], BF16, "ident")
        eshift = const.tile([128, 1], F32, tag="eshift")
        nc.vector.memset(eshift[:], ESHIFT)
        # group-0 inputs load before the bulky critic weights so the first
        # h1/qk matmuls can start ~10us earlier.
        _g0_xT = io.tile([OBS, G, 128], BF16, tag="xT")
        nc.sync.dma_start(
            out=_g0_xT[:], in_=xt_d[0:G].rearrange("e d n -> d e n")
        )
        _g0_mask = io.tile([128, G, N], F32, tag="mask_g")
        nc.sync.dma_start(
            out=_g0_mask[:], in_=mask_d[0:G].rearrange("e n m -> n e m")
        )
        _g0_state = {"xT": _g0_xT, "mask_g": _g0_mask}
        # critic weights load later - first groups' encoder work can start.
        w1t = load_const(w1t_d, [128, 2, H], BF16, "w1t")
        w1l = load_const(w1l_d, [1, H], BF16, "w1l")
        w2t = load_const(w2t_d, [128, 2, H], BF16, "w2t")
        w3t = load_const(w3t_d, [128, 2, H], F32R, "w3t")
        w4t = load_const(w4t_d, [128, 2], F32R, "w4t")
        b12 = load_const(b1_d, [128, 2], F32, "b12")
        b22 = load_const(b2_d, [128, 2], F32, "b22")
        b32 = load_const(b3_d, [128, 2], F32, "b32")
        actt = load_const(act_d, [1, BE * N], BF16, "actt")

        def seg_load(g):
            g4 = g * G
            st = {}
            xT = io.tile([OBS, G, 128], BF16, tag="xT")
            nc.sync.dma_start(
                out=xT[:], in_=xt_d[g4 : g4 + G].rearrange("e d n -> d e n")
            )
            mask_g = io.tile([128, G, N], F32, tag="mask_g")
            nc.sync.dma_start(
                out=mask_g[:], in_=mask_d[g4 : g4 + G].rearrange("e n m -> n e m")
            )
            st["mask_g"] = mask_g
            st["xT"] = xT
            return st

        def seg_h1(g, st):
            xT = st["xT"]
            h1 = wk.tile([128, 2, 512], BF16, tag="h1")
            for m in range(2):
                ps = pmm.tile([128, 512], F32, tag="mm")
                nc.tensor.matmul(
                    ps[:],
                    wet[:, m * 128 : (m + 1) * 128],
                    xT.rearrange("d e n -> d (e n)"),
                    start=True,
                    stop=True,
                )
                nc.scalar.activation(
                    h1[:, m, :], ps[:], Relu, bias=be2[:, m : m + 1], scale=1.0
                )
            st["h1"] = h1

        def _qk_layer(h1, wt, bias2, tag):
            o = wk.tile([128, 2, 512], BF16, tag=tag)
            for m in range(2):
                ps = pmm.tile([128, 512], F32, tag="mm")
                for kt in range(2):
                    nc.tensor.matmul(
                        ps[:],
                        wt[:, kt, m * 128 : (m + 1) * 128],
                        h1[:, kt, :],
                        start=(kt == 0),
                        stop=(kt == 1),
                    )
                nc.scalar.activation(
                    o[:, m, :], ps[:], Relu, bias=bias2[:, m : m + 1], scale=1.0
                )
            return o

        def seg_q(g, st):
            st["qT"] = _qk_layer(st["h1"], wqt, bq2, "qT")

        def seg_k(g, st):
            st["kT"] = _qk_layer(st["h1"], wkt, bk2, "kT")

        def seg_v(g, st):
            h1 = st["h1"]
            v_bf = wk.tile([128, G, H], BF16, tag="v_bf")
            for e in range(G):
                ps = psm.tile([128, H], F32, tag="sm")
                for kt in range(2):
                    nc.tensor.matmul(
                        ps[:],
                        h1[:, kt, e * 128 : (e + 1) * 128],
                        wvt[:, kt, :],
                        start=(kt == 0),
                        stop=False,
                    )
                nc.tensor.matmul(ps[:], ones1[:], bvrow[:], start=False, stop=True)
                nc.vector.tensor_scalar(
                    out=v_bf[:, e, :], in0=ps[:], scalar1=0.0, scalar2=None,
                    op0=Alu.max,
                )
            st["v_bf"] = v_bf

        def seg_scores(g, st):
            g4 = g * G
            qT, kT, mask_g = st["qT"], st["kT"], st["mask_g"]
            nb = wk.tile([128, G * N], F32, tag="nb")
            nc.gpsimd.tensor_scalar(
                out=nb[:], in0=mask_g.rearrange("n e m -> n (e m)"),
                scalar1=BIG, scalar2=-BIG, op0=Alu.mult, op1=Alu.add,
            )
            t_g = wk.tile([128, G, N], F32, tag="t_g")
            for e in range(G):
                sps = psm.tile([128, N], F32, tag="sm")
                for kt in range(2):
                    nc.tensor.matmul(
                        sps[:],
                        qT[:, kt, e * 128 : (e + 1) * 128],
                        kT[:, kt, e * 128 : (e + 1) * 128],
                        start=(kt == 0),
                        stop=(kt == 1),
                    )
                nc.vector.scalar_tensor_tensor(
                    out=t_g[:, e, :], in0=sps[:], scalar=0.0, in1=mask_g[:, e, :],
                    op0=Alu.max, op1=Alu.mult,
                )
            a_w = awp.tile([128, G, N], F32, tag="a_w")
            nc.gpsimd.tensor_add(
                a_w.rearrange("n e m -> n (e m)"),
                t_g.rearrange("n e m -> n (e m)"),
                nb[:],
            )
            nc.sync.dma_start(
                out=aw_d[g4 : g4 + G].rearrange("e n m -> n e m"), in_=a_w[:]
            )
            E_g = wk.tile([128, G, N], BF16, tag="E_g")
            nc.scalar.activation(
                E_g.rearrange("n e m -> n (e m)"),
                a_w.rearrange("n e m -> n (e m)"),
                Exp, bias=eshift[:], scale=1.0,
            )
            ssum = wk.tile([128, G], F32, tag="ssum")
            nc.vector.tensor_reduce(
                ssum[:], E_g[:], axis=mybir.AxisListType.X, op=Alu.add
            )
            rinv = wk.tile([128, G], F32, tag="rinv")
            nc.vector.reciprocal(rinv[:], ssum[:])
            att = wk.tile([128, G, N], BF16, tag="att")
            for e in range(G):
                nc.vector.tensor_scalar(
                    out=att[:, e, :], in0=E_g[:, e, :],
                    scalar1=rinv[:, e : e + 1], scalar2=None, op0=Alu.mult,
                )
            st["att"] = att

        def seg_attT(g, st):
            # transpose via a REGULAR matmul (att.T @ I): runs at the warm
            # 2.4GHz clock and counts as PE-busy for HAM, unlike
            # transpose-mode.
            aT_ps = pmm.tile([128, G * 128], F32, tag="mm")
            att = st["att"]
            for e in range(G):
                nc.tensor.matmul(
                    aT_ps[:, e * 128 : (e + 1) * 128], att[:, e, :], ident[:],
                    start=True, stop=True,
                )
            aT = wk.tile([128, G * 128], BF16, tag="aT")
            nc.scalar.activation(aT[:], aT_ps[:], Copy)
            st["aT"] = aT

        def seg_h2(g, st):
            v_bf, aT = st["v_bf"], st["aT"]
            h2 = wk.tile([128, 2, 512], BF16, tag="h2")
            for m in range(2):
                ps = pmm.tile([128, 512], F32, tag="mm")
                for e in range(G):
                    nc.tensor.matmul(
                        ps[:, e * 128 : (e + 1) * 128],
                        v_bf[:, e, m * 128 : (m + 1) * 128],
                        aT[:, e * 128 : (e + 1) * 128],
                        start=True,
                        stop=True,
                    )
                nc.scalar.activation(h2[:, m, :], ps[:], Copy)
            st["h2"] = h2

        def _critic_layer(g, inp, wt, bias2, tag, extra=None, act_copies=(),
                          odt=F32R):
            o = wk.tile([128, 2, 512], odt, tag=tag)
            for m in range(2):
                ps = pmm.tile([128, 512], F32, tag="mm")
                nfin = 2 if extra is None else 3
                idx = 0
                for kt in range(2):
                    idx += 1
                    nc.tensor.matmul(
                        ps[:],
                        wt[:, kt, m * 128 : (m + 1) * 128],
                        inp[:, kt, :],
                        start=(kt == 0),
                        stop=(idx == nfin),
                    )
                if extra is not None:
                    w1l_t, act_slab = extra
                    idx += 1
                    nc.tensor.matmul(
                        ps[:],
                        w1l_t[:, m * 128 : (m + 1) * 128],
                        act_slab,
                        start=False,
                        stop=(idx == nfin),
                    )
                if m in act_copies:
                    nc.scalar.activation(
                        o[:, m, :], ps[:], Relu,
                        bias=bias2[:, m : m + 1], scale=1.0,
                    )
                else:
                    nc.vector.tensor_scalar(
                        out=o[:, m, :], in0=ps[:],
                        scalar1=bias2[:, m : m + 1], scalar2=0.0,
                        op0=Alu.add, op1=Alu.max,
                    )
            return o

        def seg_c2(g, st):
            g4 = g * G
            act_slab = actt[:, g4 * N : (g4 + G) * N]
            st["c2"] = _critic_layer(
                g, st["h2"], w1t, b12, "c2", extra=(w1l, act_slab), odt=BF16
            )

        def seg_c3(g, st):
            st["c3"] = _critic_layer(g, st["c2"], w2t, b22, "c3")

        def seg_c4(g, st):
            st["c4"] = _critic_layer(g, st["c3"], w3t, b32, "c4", act_copies=(0,))

        def seg_value(g, st):
            c4 = st["c4"]
            vps = psm.tile([1, 512], F32, tag="sm")
            for kt in range(2):
                nc.tensor.matmul(
                    vps[:],
                    w4t[:, kt : kt + 1],
                    c4[:, kt, :],
                    start=(kt == 0),
                    stop=(kt == 1),
                )
            val_sb = wk.tile([1, 512], F32, tag="val_sb")
            nc.scalar.activation(val_sb[:], vps[:], Copy)
            nc.sync.dma_start(
                out=val_d[:, g * 512 : (g + 1) * 512], in_=val_sb[:]
            )

        # 3-deep software pipeline: per outer step, interleave segments of
        # groups a=t (fresh), b=t-1 (attention tail), c=t-2 (critic tail)
        # so every cross-engine handoff is covered by PE work from another
        # group.
        state = {0: _g0_state}
        for t in range(NG + 2):
            a, bq, cq = t, t - 1, t - 2
            ina = a < NG
            inb = 0 <= bq < NG
            inc_ = 0 <= cq
            if a + 1 < NG:
                state[a + 1] = seg_load(a + 1)
            if ina:
                seg_h1(a, state[a])
            if inc_:
                seg_c4(cq, state[cq])
            if inb:
                seg_attT(bq, state[bq])
            if ina:
                seg_q(a, state[a])
            if inb:
                seg_h2(bq, state[bq])
            if ina:
                seg_k(a, state[a])
            if inc_:
                seg_value(cq, state.pop(cq))
            if inb:
                seg_c2(bq, state[bq])
            if ina:
                seg_v(a, state[a])
            if inb:
                seg_c3(bq, state[bq])
            if ina:
                seg_scores(a, state[a])

    split_multi_waits(nc)
    return nc


_CACHE = {}


def _get_program():
    if "nc" not in _CACHE:
        _CACHE["nc"] = build_program()
    return _CACHE["nc"]


def _pack_kt(wT):
    """[256, out] -> [128, 2, out] with kt-major partition packing."""
    out = wT.shape[1]
    return np.ascontiguousarray(
        wT.reshape(2, 128, out).transpose(1, 0, 2)
    )


def _pack_bias(b):
    return np.ascontiguousarray(b.reshape(2, 128).T)


def kernel(**inputs):
    x = np.asarray(inputs["x"], np.float32)
    mask = np.asarray(inputs["mask"], np.float32)
    action = np.asarray(inputs["action"], np.float32)
    We = np.asarray(inputs["We"], np.float32)
    be_ = np.asarray(inputs["be"], np.float32)
    Wv = np.asarray(inputs["Wv"], np.float32)
    bv = np.asarray(inputs["bv"], np.float32)
    Wk = np.asarray(inputs["Wk"], np.float32)
    bk = np.asarray(inputs["bk"], np.float32)
    Wq = np.asarray(inputs["Wq"], np.float32)
    bq = np.asarray(inputs["bq"], np.float32)
    W1 = np.asarray(inputs["W1"], np.float32)
    b1 = np.asarray(inputs["b1"], np.float32)
    W2 = np.asarray(inputs["W2"], np.float32)
    b2 = np.asarray(inputs["b2"], np.float32)
    W3 = np.asarray(inputs["W3"], np.float32)
    b3 = np.asarray(inputs["b3"], np.float32)
    W4 = np.asarray(inputs["W4"], np.float32)
    b4 = np.asarray(inputs["b4"], np.float32)

    consts = dict(
        wet=np.ascontiguousarray(We.T).astype(NPBF),
        wqt=_pack_kt(Wq.T).astype(NPBF),
        wkt=_pack_kt(Wk.T).astype(NPBF),
        wvt=_pack_kt(Wv.T).astype(NPBF),
        bvrow=bv.reshape(1, H).astype(NPBF),
        w1t=_pack_kt(np.ascontiguousarray(W1[:, :H].T)),
        w1l=np.ascontiguousarray(W1[:, H]).reshape(1, H),
        w2t=_pack_kt(np.ascontiguousarray(W2.T)),
        w3t=_pack_kt(np.ascontiguousarray(W3.T)),
        w4t=np.ascontiguousarray(W4.reshape(256).reshape(2, 128).T),
        be2=_pack_bias(be_),
        bq2=_pack_bias(bq),
        bk2=_pack_bias(bk),
        b12=_pack_bias(b1),
        b22=_pack_bias(b2),
        b32=_pack_bias(b3),
        ident=np.eye(128, dtype=np.float32).astype(NPBF),
        ones=np.ones((1, 128), dtype=np.float32).astype(NPBF),
    )
    for k in ("w1t", "w1l", "w2t"):
        consts[k] = consts[k].astype(NPBF)
    for k in ("w3t", "w4t"):
        consts[k] = consts[k].astype(np.float32)

    xt = np.ascontiguousarray(x.transpose(0, 2, 1)).astype(NPBF)
    in_maps = []
    for c in range(NCORES):
        sl = slice(c * BE, (c + 1) * BE)
        m = dict(consts)
        m["xt"] = xt[sl]
        m["mask"] = np.ascontiguousarray(mask[sl])
        m["action"] = np.ascontiguousarray(
            action[sl].reshape(1, BE * N)
        ).astype(NPBF)
        in_maps.append(m)

    nc = _get_program()
    res = run_bass_kernel_spmd(nc, in_maps, list(range(NCORES)))
    _CACHE["last_res"] = res

    aw_parts = []
    val_parts = []
    for c in range(NCORES):
        aw_parts.append(res.results[c]["a_w"])
        val_parts.append(res.results[c]["value"].reshape(BE, N, 1))
    a_w = np.concatenate(aw_parts, axis=0)
    value = np.concatenate(val_parts, axis=0) + b4.reshape(1, 1, 1)
    return value.astype(np.float32), a_w.astype(np.float32)
